# revision 29
# baseline (speedup 1.0000x reference)
"""GCN (2x GCNConv + linear classifier) on 8 Trainium2 NeuronCores.

Strategy: shard nodes (output rows) across the 8 cores; partition edges by
destination so the scatter-add is core-local; replicate the small weight
matrices; exchange transformed source-node features with an AllGather
before each conv's aggregation.

Aggregation is done on TensorE: edges are grouped by destination into
"groups" of <=128 destinations / <=4096 edges.  For each 128-edge chunk we
build a one-hot selector S[e, d] = w[e] * (dest_local[e] == d) on VectorE
and accumulate S.T @ gathered_messages into the group's PSUM tile.  The
D^-1/2 normalizations are folded into the gather source (dinv * h) and the
epilogue (one fused tensor_scalar: scale + relu).

All heavy data moves in bf16; accumulation in f32 PSUM.
"""

import math
import os
import sys

import numpy as np

sys.path.insert(0, "/opt/trn_rl_repo")

import ml_dtypes  # noqa: E402

from concourse import bass, mybir, bacc, tile  # noqa: E402
from concourse import bass_utils  # noqa: E402

R = 8  # cores
N = 100000
F = 512
D = 256
C = 10
NSH = N // R
GROUP_EDGES = 4096  # 32 chunks of 128
BLK = 4  # dest-groups per gather call block
NQ = 4  # SWDGE queues for pipelined gathers
GCH = 8  # chunks per gather call
SP = False  # single_packet for dma_gather
MANUAL_SYNC = False  # manual qsems/wait_ge vs tile auto-sync for gathers
BF16 = mybir.dt.bfloat16
F32 = mybir.dt.float32
I16 = mybir.dt.int16


def _cdiv(a, b):
    return (a + b - 1) // b


# ----------------------------------------------------------------------------
# Host-side preprocessing: sharding, dest-group packing, gather index tables.
# Pure index manipulation / permutation of the inputs (no float arithmetic).
# ----------------------------------------------------------------------------
def _preprocess(x, edge_index, edge_weight):
    src = np.asarray(edge_index[0], dtype=np.int64)
    dst = np.asarray(edge_index[1], dtype=np.int64)
    w = np.asarray(edge_weight, dtype=np.float32)
    loop = np.arange(N, dtype=np.int64)
    src = np.concatenate([src, loop])
    dst = np.concatenate([dst, loop])
    w = np.concatenate([w, np.ones(N, np.float32)])

    cnt = np.bincount(dst, minlength=N)  # edges into each dest (incl. self)

    # --- per-core greedy dest-group packing (<=128 dests, <=GROUP_EDGES edges)
    core_groups = []  # per core: list of (start_local, end_local)
    for r in range(R):
        c = cnt[r * NSH : (r + 1) * NSH]
        gs = []
        start = 0
        tot = 0
        for i in range(NSH):
            if tot + c[i] > GROUP_EDGES or i - start == 128:
                gs.append((start, i))
                start = i
                tot = 0
            tot += c[i]
        gs.append((start, NSH))
        core_groups.append(gs)

    G = max(len(gs) for gs in core_groups)
    G = _cdiv(G, BLK) * BLK  # pad to block multiple
    assert G <= 127, f"G={G} too large for int16 gather windows"
    GP = G * 128
    W2 = 2 * GP  # rows per gather source window (2 shards)
    assert W2 - 1 <= 32767

    # --- padded numbering
    pcols = []  # per core: local node -> padded column (g*128 + slot)
    grp_ids = []
    for r in range(R):
        gs = core_groups[r]
        pcol = np.empty(NSH, np.int64)
        gid = np.empty(NSH, np.int64)
        for g, (a, b) in enumerate(gs):
            pcol[a:b] = g * 128 + np.arange(b - a)
            gid[a:b] = g
        pcols.append(pcol)
        grp_ids.append(gid)
    prow_global = np.concatenate([r * GP + pcols[r] for r in range(R)])

    # --- per-core edge cell assignment
    ecore = dst // NSH
    per_core = []
    cell_counts = np.zeros((R, G, 4), np.int64)
    for r in range(R):
        m = ecore == r
        es, ed, ew = src[m], dst[m], w[m]
        ld = ed - r * NSH
        g_of = grp_ids[r][ld]
        dl = pcols[r][ld] - g_of * 128
        psrc = prow_global[es]
        p_of = psrc // W2
        cell_counts[r] += np.zeros((G, 4), np.int64)
        np.add.at(cell_counts[r], (g_of, p_of), 1)
        per_core.append((es, ew, g_of, dl, psrc, p_of))

    n_chunks = _cdiv(cell_counts.max(axis=0), 128)  # [G, 4] uniform
    # ensure each group has at least one chunk (PSUM init)
    empty = n_chunks.sum(axis=1) == 0
    n_chunks[empty, 0] = 1

    # processing order: block-major, window-major, group-minor
    order_cells = []  # (g, p) in processing order
    call_meta = []  # per call: (p, [(g, ng)...], chunk_start)
    ch = 0
    for b0 in range(0, G, BLK):
        for p in range(4):
            gl = [(g, int(n_chunks[g, p])) for g in range(b0, b0 + BLK) if n_chunks[g, p] > 0]
            if not gl:
                continue
            call_meta.append((p, gl, ch))
            for g, ng in gl:
                order_cells.append((g, p))
                ch += ng
    TOTCH = ch

    # per-group first/last chunk rank (for PSUM start/stop flags)
    first_chunk = {}
    last_chunk = {}
    ch = 0
    for p, gl, c0 in call_meta:
        for g, ng in gl:
            if g not in first_chunk:
                first_chunk[g] = ch
            last_chunk[g] = ch + ng - 1
            ch += ng

    cell_rank = -np.ones((G, 4), np.int64)
    cell_slot_start = np.zeros(len(order_cells) + 1, np.int64)
    for i, (g, p) in enumerate(order_cells):
        cell_rank[g, p] = i
        cell_slot_start[i + 1] = cell_slot_start[i] + n_chunks[g, p] * 128

    MAXD = int(cnt.max())

    # --- per-core device arrays
    cores = []
    for r in range(R):
        es, ew, g_of, dl, psrc, p_of = per_core[r]
        key = cell_rank[g_of, p_of]
        assert (key >= 0).all()
        order = np.argsort(key, kind="stable")
        ko = key[order]
        # position within cell
        cell_edge_start = np.zeros(len(order_cells), np.int64)
        cc = np.bincount(ko, minlength=len(order_cells))
        cell_edge_start[1:] = np.cumsum(cc)[:-1]
        pos = cell_slot_start[ko] + (np.arange(len(ko)) - cell_edge_start[ko])

        slots_idx = np.zeros(TOTCH * 128, np.int16)
        slots_w = np.zeros(TOTCH * 128, np.float32)
        slots_dl = np.zeros(TOTCH * 128, np.float32)
        slots_idx[pos] = (psrc[order] % W2).astype(np.int16)
        slots_w[pos] = ew[order]
        slots_dl[pos] = dl[order].astype(np.float32)

        # idx16 wrapped per call, concatenated: [128, TOTCH*8]
        idx_parts = []
        for p, gl, c0 in call_meta:
            nch = sum(ng for _, ng in gl)
            fl = slots_idx[c0 * 128 : (c0 + nch) * 128]
            wrapped = fl.reshape(-1, 16).T  # [16, nch*8]
            idx_parts.append(np.tile(wrapped, (8, 1)))
        IDX16 = np.ascontiguousarray(np.concatenate(idx_parts, axis=1))

        DEST = np.ascontiguousarray(slots_dl.reshape(TOTCH, 128).T)
        WV = np.ascontiguousarray(slots_w.reshape(TOTCH, 128).T)

        # ELL weights for degree: [128, G*MAXD]
        pc_e = pcols[r][dst[ecore == r] - r * NSH]
        o2 = np.argsort(pc_e, kind="stable")
        pcs = pc_e[o2]
        dstart = np.zeros(GP, np.int64)
        dcnt = np.bincount(pcs, minlength=GP)
        dstart[1:] = np.cumsum(dcnt)[:-1]
        kidx = np.arange(len(pcs)) - dstart[pcs]
        WELLmat = np.zeros((GP, MAXD), np.float32)
        WELLmat[pcs, kidx] = ew[o2]
        unused = dcnt == 0
        WELLmat[unused, 0] = 1.0  # pad dests get deg=1 (avoid inf)
        WELL = np.ascontiguousarray(
            WELLmat.reshape(G, 128, MAXD).transpose(1, 0, 2).reshape(128, G * MAXD)
        )

        # padded transposed x
        xs = np.asarray(x[r * NSH : (r + 1) * NSH], np.float32)
        xT = np.zeros((F, GP), ml_dtypes.bfloat16)
        xT[:, pcols[r]] = xs.T.astype(ml_dtypes.bfloat16)
        cores.append(dict(IDX16=IDX16, DEST=DEST, WV=WV, WELL=WELL, xT=xT))

    meta = dict(
        G=G,
        GP=GP,
        W2=W2,
        MAXD=MAXD,
        TOTCH=TOTCH,
        call_meta=call_meta,
        first_chunk=first_chunk,
        last_chunk=last_chunk,
        pcols=pcols,
    )
    return cores, meta


# ----------------------------------------------------------------------------
# Device program
# ----------------------------------------------------------------------------
def _build(meta, has_b1, has_b2, has_bc, stage=99, reps=1, abl=None, collectives=True,
           shared_src=False):
    G, GP, W2, MAXD, TOTCH = (
        meta["G"],
        meta["GP"],
        meta["W2"],
        meta["MAXD"],
        meta["TOTCH"],
    )
    call_meta = meta["call_meta"]
    first_chunk, last_chunk = meta["first_chunk"], meta["last_chunk"]

    nc = bacc.Bacc(
        "TRN2",
        target_bir_lowering=False,
        debug=False,
        enable_asserts=False,
        num_devices=R,
        num_swdge_queues=NQ,
        dynamic_dma_scratch_size=32768,
    )
    dt_in = lambda name, shape, dt: nc.dram_tensor(name, shape, dt, kind="ExternalInput").ap()
    xT_d = dt_in("xT", [F, GP], BF16)
    W1_d = dt_in("W1", [F, D], F32)
    W2_d = dt_in("W2m", [D, D], F32)
    Wc_d = dt_in("Wc", [D, C], F32)
    b1_d = dt_in("b1b", [128, D], F32) if has_b1 else None
    b2_d = dt_in("b2b", [128, D], F32) if has_b2 else None
    bc_d = dt_in("bcb", [128, C], F32) if has_bc else None
    IDX_d = dt_in("IDX16", [128, TOTCH * 8], I16)
    DEST_d = dt_in("DEST", [128, TOTCH], F32)
    WV_d = dt_in("WV", [128, TOTCH], F32)
    WELL_d = dt_in("WELL", [128, G * MAXD], F32)
    IOTA_d = dt_in("IOTA", [128, 128], BF16)
    IOTAC_d = dt_in("IOTAC", [128, 1], F32)
    out_d = nc.dram_tensor("out", [GP, C], F32, kind="ExternalOutput").ap()

    AluOp = mybir.AluOpType
    replica = [list(range(R))]

    qsems = [nc.alloc_semaphore(f"gq{q}") for q in range(NQ)]
    qstate = {"next": 0, "cnt": [0] * NQ}

    with tile.TileContext(nc) as tc:
        with tc.tile_pool(name="const", bufs=1) as cpool, \
             tc.tile_pool(name="dram", bufs=1, space="DRAM") as dr:
            # ---- resident constants
            W1bf = cpool.tile([128, 4 * D], BF16)
            for b in range(4):
                nc.gpsimd.dma_start(W1bf[:, b * D : (b + 1) * D], W1_d[b * 128 : (b + 1) * 128, :])
            W2bf = cpool.tile([128, 2 * D], BF16)
            for b in range(2):
                nc.gpsimd.dma_start(W2bf[:, b * D : (b + 1) * D], W2_d[b * 128 : (b + 1) * 128, :])
            Wcbf = cpool.tile([128, 2 * C], BF16)
            for b in range(2):
                nc.gpsimd.dma_start(Wcbf[:, b * C : (b + 1) * C], Wc_d[b * 128 : (b + 1) * 128, :])
            IOTA_sb = cpool.tile([128, 128], BF16)
            nc.sync.dma_start(IOTA_sb[:], IOTA_d[:])
            iotac = cpool.tile([128, 1], F32)
            nc.sync.dma_start(iotac[:], IOTAC_d[:])
            ident = cpool.tile([128, 128], BF16)
            nc.vector.tensor_scalar(
                out=ident[:], in0=IOTA_sb[:], scalar1=iotac[:], scalar2=None,
                op0=AluOp.is_equal,
            )
            bias_tiles = {}
            for nm, d_ap, has in (("b1", b1_d, has_b1), ("b2", b2_d, has_b2), ("bc", bc_d, has_bc)):
                if has:
                    t = cpool.tile([128, d_ap.shape[1]], F32, name=f"bias_{nm}")
                    nc.sync.dma_start(t[:], d_ap[:])
                    bias_tiles[nm] = t

            # ---- degrees -> r = 1/deg, dinv = 1/sqrt(deg)
            deg = cpool.tile([128, G], F32)
            r_ = cpool.tile([128, G], F32)
            dinv = cpool.tile([128, G], F32)
            with tc.tile_pool(name="well", bufs=1) as wpool:
                well_sb = wpool.tile([128, G * MAXD], F32)
                nc.sync.dma_start(well_sb[:], WELL_d[:])
                nc.vector.tensor_reduce(
                    out=deg[:],
                    in_=well_sb[:].rearrange("p (g m) -> p g m", m=MAXD),
                    axis=mybir.AxisListType.X,
                    op=AluOp.add,
                )
            nc.vector.reciprocal(r_[:], deg[:])
            nc.scalar.sqrt(dinv[:], r_[:])

            for _rep in range(reps):
              # ---- DRAM buffers (per rep: Shared tiles allow one writer)
              spc = dict(addr_space="Shared") if shared_src else {}
              AGIN0 = dr.tile([GP, D], BF16, name=f"AGIN0_{_rep}")
              H0F = dr.tile([R * GP, D], BF16, name=f"H0F_{_rep}", **spc)
              H1BUF = dr.tile([GP, D], BF16, name=f"H1BUF_{_rep}")
              H1F = dr.tile([R * GP, D], BF16, name=f"H1F_{_rep}", **spc)
              # ---- stage B: h0 = dinv * (x @ W1), write to AGIN0
              NB = 8
              with tc.tile_pool(name="xg", bufs=2) as xpool, \
                   tc.tile_pool(name="bps", bufs=4, space="PSUM") as bps, \
                   tc.tile_pool(name="hstage", bufs=4) as hpool:
                  for gc in range(0, G, NB):
                      nb = min(NB, G - gc)
                      xg = xpool.tile([128, 4 * nb * 128], BF16, tag="xg")
                      for b in range(4):
                          nc.sync.dma_start(
                              xg[:, b * nb * 128 : (b + 1) * nb * 128],
                              xT_d[b * 128 : (b + 1) * 128, gc * 128 : (gc + nb) * 128],
                          )
                      for gi in range(nb):
                          g = gc + gi
                          pt = bps.tile([128, D], F32, tag="bpt")
                          for b in range(4):
                              nc.tensor.matmul(
                                  out=pt[:],
                                  lhsT=xg[:, (b * nb + gi) * 128 : (b * nb + gi + 1) * 128],
                                  rhs=W1bf[:, b * D : (b + 1) * D],
                                  start=(b == 0),
                                  stop=(b == 3),
                              )
                          h0 = hpool.tile([128, D], BF16, tag="h0")
                          nc.vector.tensor_scalar(
                              out=h0[:], in0=pt[:], scalar1=dinv[:, g : g + 1],
                              scalar2=None, op0=AluOp.mult,
                          )
                          nc.sync.dma_start(AGIN0[g * 128 : (g + 1) * 128, :], h0[:])

              # ---- AG1
              if stage >= 2 and collectives:
                  nc.gpsimd.collective_compute(
                      "AllGather", AluOp.bypass, replica_groups=replica,
                      ins=[AGIN0.opt()], outs=[H0F.opt()],
                  )

              # ---- conv aggregation (shared for conv1 / conv2)
              def aggregate(src_full, scal, bias_tile, out_cb, psum_bufs=8, act=True):
                  """scatter-add aggregation over all groups.

                  src_full: [R*GP, D] gather source; scal: [128,G] per-dest scale
                  applied pre-relu; out_cb(g, tile_bf16) consumes the result.
                  """
                  with tc.tile_pool(name="agg_ps", bufs=psum_bufs, space="PSUM") as aps, \
                       tc.tile_pool(name="gat", bufs=10) as gpool, \
                       tc.tile_pool(name="meta", bufs=4) as mpool, \
                       tc.tile_pool(name="sel", bufs=12) as spool, \
                       tc.tile_pool(name="hout", bufs=4) as opool, \
                       tc.tile_pool(name="epi", bufs=2) as epool:
                      pst = {}
                      ch = 0
                      for ci, (p, gl, c0) in enumerate(call_meta):
                          nch = sum(ng for _, ng in gl)
                          idx_sb = mpool.tile([128, nch * 8], I16, tag="idx")
                          nc.sync.dma_start(idx_sb[:], IDX_d[:, c0 * 8 : (c0 + nch) * 8])
                          dest_sb = mpool.tile([128, nch], F32, tag="dst")
                          nc.sync.dma_start(dest_sb[:], DEST_d[:, c0 : c0 + nch])
                          wv_sb = mpool.tile([128, nch], F32, tag="wv")
                          nc.sync.dma_start(wv_sb[:], WV_d[:, c0 : c0 + nch])
                          # pipelined sub-gathers: prepare_only + trigger on
                          # round-robin SWDGE queues; consumers wait manually
                          # on the queue's DMA-completion sem (tile's auto
                          # waits under-sync multi-prep pipelines).
                          # probe ablations: ehalf gathers 256B elems, edouble 1024B
                          EL = D // 2 if abl == "ehalf" else (2 * D if abl == "edouble" else D)
                          win = src_full[p * W2 : (p + 1) * W2, :]
                          if abl == "ehalf":
                              win = win.rearrange("a (two d) -> (a two) d", two=2)
                          elif abl == "edouble":
                              win = win.rearrange("(a two) d -> a (two d)", two=2)
                          subs = []  # per sub-gather: (tile, a, b, q, tick)
                          if abl != "compute":
                              for a in range(0, nch, GCH):
                                  b = min(a + GCH, nch)
                                  Msub = gpool.tile([128, (b - a) * EL], BF16, tag="gat")
                                  q = qstate["next"]
                                  qstate["next"] = (q + 1) % NQ
                                  qstate["cnt"][q] += 1
                                  gi = nc.gpsimd.dma_gather(
                                      out_ap=Msub[:].rearrange("p (t d) -> p t d", d=EL),
                                      in_ap=win,
                                      idxs_ap=idx_sb[:, a * 8 : b * 8],
                                      num_idxs=(b - a) * 128,
                                      num_idxs_reg=(b - a) * 128,
                                      elem_size=EL,
                                      single_packet=SP,
                                      queue_num=q,
                                  )
                                  if MANUAL_SYNC:
                                      gi.then_inc(qsems[q], 16)
                                  subs.append((Msub, a, b, q, qstate["cnt"][q]))
                          if abl in ("gather", "compute", "ehalf", "edouble"):
                              continue
                          t = 0
                          si = -1
                          Mcur = None
                          for g, ng in gl:
                              if g not in pst:
                                  pst[g] = aps.tile([128, D], F32, tag="agg", name=f"agg{g}")
                              for k in range(ng):
                                  if si < 0 or t >= subs[si][2]:
                                      si += 1
                                      Mcur = subs[si][0]
                                      if MANUAL_SYNC:
                                          nc.tensor.wait_ge(qsems[subs[si][3]], 16 * subs[si][4])
                                  if abl == "nosel":
                                      S = ident
                                  else:
                                      S = spool.tile([128, 128], BF16, tag="sel")
                                      nc.vector.tensor_scalar(
                                          out=S[:], in0=IOTA_sb[:],
                                          scalar1=dest_sb[:, t : t + 1],
                                          scalar2=wv_sb[:, t : t + 1],
                                          op0=AluOp.is_equal, op1=AluOp.mult,
                                      )
                                  nc.tensor.matmul(
                                      out=pst[g][:],
                                      lhsT=S[:],
                                      rhs=Mcur[:, (t - subs[si][1]) * D : (t - subs[si][1] + 1) * D],
                                      start=(ch == first_chunk[g]),
                                      stop=(ch == last_chunk[g]),
                                  )
                                  t += 1
                                  ch += 1
                          # epilogue for any group whose last chunk has been issued
                          done = [g for g in list(pst) if last_chunk[g] < ch]
                          for g in done:
                              h_t = opool.tile([128, D], BF16, tag="h")
                              if not act:
                                  nc.scalar.activation(
                                      out=h_t[:], in_=pst[g][:],
                                      func=mybir.ActivationFunctionType.Copy,
                                      scale=scal[:, g : g + 1],
                                  )
                              elif bias_tile is None:
                                  nc.scalar.activation(
                                      out=h_t[:], in_=pst[g][:],
                                      func=mybir.ActivationFunctionType.Relu,
                                      scale=scal[:, g : g + 1],
                                  )
                              else:
                                  tmp = epool.tile([128, D], F32, tag="tmp")
                                  nc.vector.tensor_scalar(
                                      out=tmp[:], in0=pst[g][:],
                                      scalar1=scal[:, g : g + 1], scalar2=None,
                                      op0=AluOp.mult,
                                  )
                                  nc.vector.tensor_tensor(
                                      out=tmp[:], in0=tmp[:], in1=bias_tile[:], op=AluOp.add
                                  )
                                  nc.vector.tensor_scalar(
                                      out=h_t[:], in0=tmp[:], scalar1=0.0,
                                      scalar2=None, op0=AluOp.max,
                                  )
                              out_cb(g, h_t)
                              del pst[g]

              # conv1: h1 = relu(r * agg + b1)  [r = dinv^2], store h1 to H1BUF
              def out1(g, h):
                  nc.sync.dma_start(H1BUF[g * 128 : (g + 1) * 128, :], h[:])

              if stage >= 3:
                  aggregate(H0F, r_, bias_tiles.get("b1"), out1)

              # ---- AG2: exchange h1' directly (W2 applied post-aggregation)
              if stage >= 5 and collectives:
                  nc.gpsimd.collective_compute(
                      "AllGather", AluOp.bypass, replica_groups=replica,
                      ins=[H1BUF.opt()], outs=[H1F.opt()],
                  )

              # ---- conv2 aggregation + W2 + classifier fused
              with tc.tile_pool(name="w2_ps", bufs=1, space="PSUM") as w2ps, \
                   tc.tile_pool(name="clf_ps", bufs=1, space="PSUM") as clps, \
                   tc.tile_pool(name="tp_ps", bufs=1, space="PSUM") as tpps, \
                   tc.tile_pool(name="h2t", bufs=4) as h2tp, \
                   tc.tile_pool(name="h2s", bufs=2) as h2sp, \
                   tc.tile_pool(name="clf_out", bufs=4) as clop:

                  def out2(g, t_bf):
                      # t_bf = dinv_g * agg2  (bf16, no act). h2 = relu(t @ W2);
                      # out = h2 @ Wc + bc
                      pt = w2ps.tile([128, D], F32, tag="w2o")
                      for b in range(2):
                          tp = tpps.tile([128, 128], BF16, tag="tp")
                          nc.tensor.transpose(
                              out=tp[:], in_=t_bf[:, b * 128 : (b + 1) * 128], identity=ident[:]
                          )
                          tT = h2tp.tile([128, 128], BF16, tag="tT")
                          nc.scalar.activation(
                              out=tT[:], in_=tp[:],
                              func=mybir.ActivationFunctionType.Copy,
                          )
                          nc.tensor.matmul(
                              out=pt[:],
                              lhsT=tT[:],
                              rhs=W2bf[:, b * D : (b + 1) * D],
                              start=(b == 0),
                              stop=(b == 1),
                          )
                      h2 = h2sp.tile([128, D], BF16, tag="h2")
                      if "b2" in bias_tiles:
                          tmp2 = h2sp.tile([128, D], F32, tag="tmp2")
                          nc.vector.tensor_tensor(
                              out=tmp2[:], in0=pt[:], in1=bias_tiles["b2"][:], op=AluOp.add
                          )
                          nc.vector.tensor_scalar(
                              out=h2[:], in0=tmp2[:], scalar1=0.0, scalar2=None, op0=AluOp.max
                          )
                      else:
                          nc.scalar.activation(
                              out=h2[:], in_=pt[:],
                              func=mybir.ActivationFunctionType.Relu,
                          )
                      cl = clps.tile([128, C], F32, tag="clf")
                      for b in range(2):
                          tp = tpps.tile([128, 128], BF16, tag="tp")
                          nc.tensor.transpose(
                              out=tp[:], in_=h2[:, b * 128 : (b + 1) * 128], identity=ident[:]
                          )
                          h2T = h2tp.tile([128, 128], BF16, tag="tT")
                          nc.scalar.activation(
                              out=h2T[:], in_=tp[:],
                              func=mybir.ActivationFunctionType.Copy,
                          )
                          nc.tensor.matmul(
                              out=cl[:],
                              lhsT=h2T[:],
                              rhs=Wcbf[:, b * C : (b + 1) * C],
                              start=(b == 0),
                              stop=(b == 1),
                          )
                      co = clop.tile([128, C], F32, tag="co")
                      if "bc" in bias_tiles:
                          nc.vector.tensor_tensor(
                              out=co[:], in0=cl[:], in1=bias_tiles["bc"][:], op=AluOp.add
                          )
                      else:
                          nc.scalar.activation(
                              out=co[:], in_=cl[:],
                              func=mybir.ActivationFunctionType.Copy,
                          )
                      nc.sync.dma_start(out_d[g * 128 : (g + 1) * 128, :], co[:])

                  if stage >= 6:
                      aggregate(H1F, dinv, None, out2, psum_bufs=5, act=False)

    nc.compile()
    return nc


# ----------------------------------------------------------------------------
# Entry point
# ----------------------------------------------------------------------------
_CACHE = {}


def _get_compiled(x, edge_index, edge_weight, b1, b2, bc):
    cores, meta = _preprocess(x, edge_index, edge_weight)
    has_b1 = bool(np.any(b1))
    has_b2 = bool(np.any(b2))
    has_bc = bool(np.any(bc))
    key = repr((meta["G"], meta["MAXD"], meta["TOTCH"], has_b1, has_b2, has_bc, meta["call_meta"]))
    nc = _CACHE.get(key)
    if nc is None:
        nc = _build(meta, has_b1, has_b2, has_bc)
        _CACHE[key] = nc
    return nc, cores, meta


def make_in_maps(inputs, cores, meta):
    x = np.asarray(inputs["x"], np.float32)
    W1 = np.asarray(inputs["W1"], np.float32)
    W2m = np.asarray(inputs["W2"], np.float32)
    Wc = np.asarray(inputs["Wc"], np.float32)
    b1 = np.asarray(inputs["b1"], np.float32)
    b2 = np.asarray(inputs["b2"], np.float32)
    bc = np.asarray(inputs["bc"], np.float32)
    iota = np.broadcast_to(np.arange(128, dtype=np.float32), (128, 128))
    in_maps = []
    for r in range(R):
        m = dict(
            xT=cores[r]["xT"],
            W1=W1,
            W2m=W2m,
            Wc=Wc,
            IDX16=cores[r]["IDX16"],
            DEST=cores[r]["DEST"],
            WV=cores[r]["WV"],
            WELL=cores[r]["WELL"],
            IOTA=np.ascontiguousarray(iota).astype(ml_dtypes.bfloat16),
            IOTAC=np.arange(128, dtype=np.float32).reshape(128, 1),
        )
        if np.any(b1):
            m["b1b"] = np.ascontiguousarray(np.broadcast_to(b1, (128, D))).astype(np.float32)
        if np.any(b2):
            m["b2b"] = np.ascontiguousarray(np.broadcast_to(b2, (128, D))).astype(np.float32)
        if np.any(bc):
            m["bcb"] = np.ascontiguousarray(np.broadcast_to(bc, (128, C))).astype(np.float32)
        in_maps.append(m)
    return in_maps


def unpack_output(results, meta):
    out = np.empty((N, C), np.float32)
    for r in range(R):
        out[r * NSH : (r + 1) * NSH] = results[r]["out"][meta["pcols"][r]]
    return out


def kernel(**inputs):
    nc, cores, meta = _get_compiled(
        inputs["x"], inputs["edge_index"], inputs["edge_weight"],
        inputs["b1"], inputs["b2"], inputs["bc"],
    )
    in_maps = make_in_maps(inputs, cores, meta)
    res = bass_utils.run_bass_kernel_spmd(nc, in_maps, core_ids=list(range(R)))
    return unpack_output(res.results, meta)



# revision 33
# speedup vs baseline: 1.3312x; 1.3312x over previous
"""GCN (2x GCNConv + linear classifier) on 8 Trainium2 NeuronCores.

Strategy: shard nodes (output rows) across the 8 cores; partition edges by
destination so the scatter-add is core-local; replicate the small weight
matrices; exchange transformed source-node features with an AllGather
before each conv's aggregation.

Aggregation is done on TensorE: edges are grouped by destination into
"groups" of <=128 destinations / <=4096 edges.  For each 128-edge chunk we
build a one-hot selector S[e, d] = w[e] * (dest_local[e] == d) on VectorE
and accumulate S.T @ gathered_messages into the group's PSUM tile.  The
D^-1/2 normalizations are folded into the gather source (dinv * h) and the
epilogue (one fused tensor_scalar: scale + relu).

All heavy data moves in bf16; accumulation in f32 PSUM.
"""

import math
import os
import sys

import numpy as np

sys.path.insert(0, "/opt/trn_rl_repo")

import ml_dtypes  # noqa: E402

from concourse import bass, mybir, bacc, tile  # noqa: E402
from concourse import bass_utils  # noqa: E402

R = 8  # cores
N = 100000
F = 512
D = 256
C = 10
NSH = N // R
GROUP_EDGES = 4096  # 32 chunks of 128
BLK = 4  # dest-groups per gather call block
NQ = 4  # SWDGE queues for pipelined gathers
GCH = 8  # chunks per gather call
SP = False  # single_packet for dma_gather
MANUAL_SYNC = False  # manual qsems/wait_ge vs tile auto-sync for gathers
BF16 = mybir.dt.bfloat16
F32 = mybir.dt.float32
I16 = mybir.dt.int16


def _cdiv(a, b):
    return (a + b - 1) // b


# ----------------------------------------------------------------------------
# Host-side preprocessing: sharding, dest-group packing, gather index tables.
# Pure index manipulation / permutation of the inputs (no float arithmetic).
# ----------------------------------------------------------------------------
def _preprocess(x, edge_index, edge_weight):
    src = np.asarray(edge_index[0], dtype=np.int64)
    dst = np.asarray(edge_index[1], dtype=np.int64)
    w = np.asarray(edge_weight, dtype=np.float32)
    loop = np.arange(N, dtype=np.int64)
    src = np.concatenate([src, loop])
    dst = np.concatenate([dst, loop])
    w = np.concatenate([w, np.ones(N, np.float32)])

    cnt = np.bincount(dst, minlength=N)  # edges into each dest (incl. self)

    # --- per-core greedy dest-group packing (<=128 dests, <=GROUP_EDGES edges)
    core_groups = []  # per core: list of (start_local, end_local)
    for r in range(R):
        c = cnt[r * NSH : (r + 1) * NSH]
        gs = []
        start = 0
        tot = 0
        for i in range(NSH):
            if tot + c[i] > GROUP_EDGES or i - start == 128:
                gs.append((start, i))
                start = i
                tot = 0
            tot += c[i]
        gs.append((start, NSH))
        core_groups.append(gs)

    G = max(len(gs) for gs in core_groups)
    G = _cdiv(G, BLK) * BLK  # pad to block multiple
    assert G <= 127, f"G={G} too large for int16 gather windows"
    GP = G * 128
    W2 = 2 * GP  # rows per gather source window (2 shards)
    assert W2 - 1 <= 32767

    # --- padded numbering
    pcols = []  # per core: local node -> padded column (g*128 + slot)
    grp_ids = []
    for r in range(R):
        gs = core_groups[r]
        pcol = np.empty(NSH, np.int64)
        gid = np.empty(NSH, np.int64)
        for g, (a, b) in enumerate(gs):
            pcol[a:b] = g * 128 + np.arange(b - a)
            gid[a:b] = g
        pcols.append(pcol)
        grp_ids.append(gid)
    prow_global = np.concatenate([r * GP + pcols[r] for r in range(R)])

    # --- per-core edge cell assignment
    ecore = dst // NSH
    per_core = []
    cell_counts = np.zeros((R, G, 4), np.int64)
    for r in range(R):
        m = ecore == r
        es, ed, ew = src[m], dst[m], w[m]
        ld = ed - r * NSH
        g_of = grp_ids[r][ld]
        dl = pcols[r][ld] - g_of * 128
        psrc = prow_global[es]
        p_of = psrc // W2
        cell_counts[r] += np.zeros((G, 4), np.int64)
        np.add.at(cell_counts[r], (g_of, p_of), 1)
        per_core.append((es, ew, g_of, dl, psrc, p_of))

    n_chunks = _cdiv(cell_counts.max(axis=0), 128)  # [G, 4] uniform
    # ensure each group has at least one chunk (PSUM init)
    empty = n_chunks.sum(axis=1) == 0
    n_chunks[empty, 0] = 1

    # processing order: block-major, window-major, group-minor
    order_cells = []  # (g, p) in processing order
    call_meta = []  # per call: (p, [(g, ng)...], chunk_start)
    ch = 0
    for b0 in range(0, G, BLK):
        for p in range(4):
            gl = [(g, int(n_chunks[g, p])) for g in range(b0, b0 + BLK) if n_chunks[g, p] > 0]
            if not gl:
                continue
            call_meta.append((p, gl, ch))
            for g, ng in gl:
                order_cells.append((g, p))
                ch += ng
    TOTCH = ch

    # per-group first/last chunk rank (for PSUM start/stop flags)
    first_chunk = {}
    last_chunk = {}
    ch = 0
    for p, gl, c0 in call_meta:
        for g, ng in gl:
            if g not in first_chunk:
                first_chunk[g] = ch
            last_chunk[g] = ch + ng - 1
            ch += ng

    cell_rank = -np.ones((G, 4), np.int64)
    cell_slot_start = np.zeros(len(order_cells) + 1, np.int64)
    for i, (g, p) in enumerate(order_cells):
        cell_rank[g, p] = i
        cell_slot_start[i + 1] = cell_slot_start[i] + n_chunks[g, p] * 128

    MAXD = int(cnt.max())

    # --- per-core device arrays
    cores = []
    for r in range(R):
        es, ew, g_of, dl, psrc, p_of = per_core[r]
        key = cell_rank[g_of, p_of]
        assert (key >= 0).all()
        order = np.argsort(key, kind="stable")
        ko = key[order]
        # position within cell
        cell_edge_start = np.zeros(len(order_cells), np.int64)
        cc = np.bincount(ko, minlength=len(order_cells))
        cell_edge_start[1:] = np.cumsum(cc)[:-1]
        pos = cell_slot_start[ko] + (np.arange(len(ko)) - cell_edge_start[ko])

        slots_idx = np.zeros(TOTCH * 128, np.int16)
        slots_w = np.zeros(TOTCH * 128, np.float32)
        slots_dl = np.zeros(TOTCH * 128, np.float32)
        slots_idx[pos] = (psrc[order] % W2).astype(np.int16)
        slots_w[pos] = ew[order]
        slots_dl[pos] = dl[order].astype(np.float32)

        # idx16 wrapped per call, concatenated: [128, TOTCH*8]
        idx_parts = []
        for p, gl, c0 in call_meta:
            nch = sum(ng for _, ng in gl)
            fl = slots_idx[c0 * 128 : (c0 + nch) * 128]
            wrapped = fl.reshape(-1, 16).T  # [16, nch*8]
            idx_parts.append(np.tile(wrapped, (8, 1)))
        IDX16 = np.ascontiguousarray(np.concatenate(idx_parts, axis=1))

        DEST = np.ascontiguousarray(slots_dl.reshape(TOTCH, 128).T)
        WV = np.ascontiguousarray(slots_w.reshape(TOTCH, 128).T)

        # ELL weights for degree: [128, G*MAXD]
        pc_e = pcols[r][dst[ecore == r] - r * NSH]
        o2 = np.argsort(pc_e, kind="stable")
        pcs = pc_e[o2]
        dstart = np.zeros(GP, np.int64)
        dcnt = np.bincount(pcs, minlength=GP)
        dstart[1:] = np.cumsum(dcnt)[:-1]
        kidx = np.arange(len(pcs)) - dstart[pcs]
        WELLmat = np.zeros((GP, MAXD), np.float32)
        WELLmat[pcs, kidx] = ew[o2]
        unused = dcnt == 0
        WELLmat[unused, 0] = 1.0  # pad dests get deg=1 (avoid inf)
        WELL = np.ascontiguousarray(
            WELLmat.reshape(G, 128, MAXD).transpose(1, 0, 2).reshape(128, G * MAXD)
        )

        # padded transposed x
        xs = np.asarray(x[r * NSH : (r + 1) * NSH], np.float32)
        xT = np.zeros((F, GP), ml_dtypes.bfloat16)
        xT[:, pcols[r]] = xs.T.astype(ml_dtypes.bfloat16)
        cores.append(dict(IDX16=IDX16, DEST=DEST, WV=WV, WELL=WELL, xT=xT))

    meta = dict(
        G=G,
        GP=GP,
        W2=W2,
        MAXD=MAXD,
        TOTCH=TOTCH,
        call_meta=call_meta,
        first_chunk=first_chunk,
        last_chunk=last_chunk,
        pcols=pcols,
    )
    return cores, meta


# ----------------------------------------------------------------------------
# Device program
# ----------------------------------------------------------------------------
def _build(meta, has_b1, has_b2, has_bc, stage=99, reps=1, abl=None, collectives=True,
           shared_src=False):
    G, GP, W2, MAXD, TOTCH = (
        meta["G"],
        meta["GP"],
        meta["W2"],
        meta["MAXD"],
        meta["TOTCH"],
    )
    call_meta = meta["call_meta"]
    first_chunk, last_chunk = meta["first_chunk"], meta["last_chunk"]

    nc = bacc.Bacc(
        "TRN2",
        target_bir_lowering=False,
        debug=False,
        enable_asserts=False,
        num_devices=R,
        num_swdge_queues=NQ,
        dynamic_dma_scratch_size=32768,
    )
    dt_in = lambda name, shape, dt: nc.dram_tensor(name, shape, dt, kind="ExternalInput").ap()
    xT_d = dt_in("xT", [F, GP], BF16)
    W1_d = dt_in("W1", [F, D], F32)
    W2_d = dt_in("W2m", [D, D], F32)
    Wc_d = dt_in("Wc", [D, C], F32)
    b1_d = dt_in("b1b", [128, D], F32) if has_b1 else None
    b2_d = dt_in("b2b", [128, D], F32) if has_b2 else None
    bc_d = dt_in("bcb", [128, C], F32) if has_bc else None
    IDX_d = dt_in("IDX16", [128, TOTCH * 8], I16)
    DEST_d = dt_in("DEST", [128, TOTCH], F32)
    WV_d = dt_in("WV", [128, TOTCH], F32)
    WELL_d = dt_in("WELL", [128, G * MAXD], F32)
    IOTA_d = dt_in("IOTA", [128, 128], BF16)
    IOTAC_d = dt_in("IOTAC", [128, 1], F32)
    out_d = nc.dram_tensor("out", [GP, C], F32, kind="ExternalOutput").ap()

    AluOp = mybir.AluOpType
    replica = [list(range(R))]

    qsems = [nc.alloc_semaphore(f"gq{q}") for q in range(NQ)]
    qstate = {"next": 0, "cnt": [0] * NQ}

    with tile.TileContext(nc) as tc:
        with tc.tile_pool(name="const", bufs=1) as cpool, \
             tc.tile_pool(name="dram", bufs=1, space="DRAM") as dr:
            # ---- resident constants
            W1bf = cpool.tile([128, 4 * D], BF16)
            for b in range(4):
                nc.gpsimd.dma_start(W1bf[:, b * D : (b + 1) * D], W1_d[b * 128 : (b + 1) * 128, :])
            W2bf = cpool.tile([128, 2 * D], BF16)
            for b in range(2):
                nc.gpsimd.dma_start(W2bf[:, b * D : (b + 1) * D], W2_d[b * 128 : (b + 1) * 128, :])
            Wcbf = cpool.tile([128, 2 * C], BF16)
            for b in range(2):
                nc.gpsimd.dma_start(Wcbf[:, b * C : (b + 1) * C], Wc_d[b * 128 : (b + 1) * 128, :])
            IOTA_sb = cpool.tile([128, 128], BF16)
            nc.sync.dma_start(IOTA_sb[:], IOTA_d[:])
            iotac = cpool.tile([128, 1], F32)
            nc.sync.dma_start(iotac[:], IOTAC_d[:])
            ident = cpool.tile([128, 128], BF16)
            nc.vector.tensor_scalar(
                out=ident[:], in0=IOTA_sb[:], scalar1=iotac[:], scalar2=None,
                op0=AluOp.is_equal,
            )
            bias_tiles = {}
            for nm, d_ap, has in (("b1", b1_d, has_b1), ("b2", b2_d, has_b2), ("bc", bc_d, has_bc)):
                if has:
                    t = cpool.tile([128, d_ap.shape[1]], F32, name=f"bias_{nm}")
                    nc.sync.dma_start(t[:], d_ap[:])
                    bias_tiles[nm] = t

            # ---- degrees -> r = 1/deg, dinv = 1/sqrt(deg)
            deg = cpool.tile([128, G], F32)
            r_ = cpool.tile([128, G], F32)
            dinv = cpool.tile([128, G], F32)
            with tc.tile_pool(name="well", bufs=1) as wpool:
                well_sb = wpool.tile([128, G * MAXD], F32)
                nc.sync.dma_start(well_sb[:], WELL_d[:])
                nc.vector.tensor_reduce(
                    out=deg[:],
                    in_=well_sb[:].rearrange("p (g m) -> p g m", m=MAXD),
                    axis=mybir.AxisListType.X,
                    op=AluOp.add,
                )
            nc.vector.reciprocal(r_[:], deg[:])
            nc.scalar.sqrt(dinv[:], r_[:])

            for _rep in range(reps):
              # ---- DRAM buffers (per rep: Shared tiles allow one writer)
              spc = dict(addr_space="Shared") if shared_src else {}
              AGIN0 = dr.tile([GP, D], BF16, name=f"AGIN0_{_rep}")
              H0F = dr.tile([R * GP, D], BF16, name=f"H0F_{_rep}", **spc)
              H1BUF = dr.tile([GP, D], BF16, name=f"H1BUF_{_rep}")
              H1F = dr.tile([R * GP, D], BF16, name=f"H1F_{_rep}", **spc)
              # ---- stage B: h0 = dinv * (x @ W1), write to AGIN0
              NB = 8
              with tc.tile_pool(name="xg", bufs=2) as xpool, \
                   tc.tile_pool(name="bps", bufs=4, space="PSUM") as bps, \
                   tc.tile_pool(name="hstage", bufs=4) as hpool:
                  for gc in range(0, G, NB):
                      nb = min(NB, G - gc)
                      xg = xpool.tile([128, 4 * nb * 128], BF16, tag="xg")
                      for b in range(4):
                          nc.sync.dma_start(
                              xg[:, b * nb * 128 : (b + 1) * nb * 128],
                              xT_d[b * 128 : (b + 1) * 128, gc * 128 : (gc + nb) * 128],
                          )
                      for gi in range(nb):
                          g = gc + gi
                          pt = bps.tile([128, D], F32, tag="bpt")
                          for b in range(4):
                              nc.tensor.matmul(
                                  out=pt[:],
                                  lhsT=xg[:, (b * nb + gi) * 128 : (b * nb + gi + 1) * 128],
                                  rhs=W1bf[:, b * D : (b + 1) * D],
                                  start=(b == 0),
                                  stop=(b == 3),
                              )
                          h0 = hpool.tile([128, D], BF16, tag="h0")
                          nc.vector.tensor_scalar(
                              out=h0[:], in0=pt[:], scalar1=dinv[:, g : g + 1],
                              scalar2=None, op0=AluOp.mult,
                          )
                          nc.sync.dma_start(AGIN0[g * 128 : (g + 1) * 128, :], h0[:])

              # ---- AG1
              if stage >= 2 and collectives:
                  nc.gpsimd.collective_compute(
                      "AllGather", AluOp.bypass, replica_groups=replica,
                      ins=[AGIN0.opt()], outs=[H0F.opt()],
                  )

              # ---- conv aggregation (shared for conv1 / conv2)
              def aggregate(src_full, scal, bias_tile, out_cb, psum_bufs=8, act=True):
                  """scatter-add aggregation over all groups.

                  src_full: [R*GP, D] gather source; scal: [128,G] per-dest scale
                  applied pre-relu; out_cb(g, tile_bf16) consumes the result.
                  """
                  with tc.tile_pool(name="agg_ps", bufs=psum_bufs, space="PSUM") as aps, \
                       tc.tile_pool(name="gat", bufs=10) as gpool, \
                       tc.tile_pool(name="meta", bufs=4) as mpool, \
                       tc.tile_pool(name="sel", bufs=12) as spool, \
                       tc.tile_pool(name="hout", bufs=4) as opool, \
                       tc.tile_pool(name="epi", bufs=2) as epool:
                      pst = {}
                      ch = 0
                      for ci, (p, gl, c0) in enumerate(call_meta):
                          nch = sum(ng for _, ng in gl)
                          idx_sb = mpool.tile([128, nch * 8], I16, tag="idx")
                          nc.sync.dma_start(idx_sb[:], IDX_d[:, c0 * 8 : (c0 + nch) * 8])
                          dest_sb = mpool.tile([128, nch], F32, tag="dst")
                          nc.sync.dma_start(dest_sb[:], DEST_d[:, c0 : c0 + nch])
                          wv_sb = mpool.tile([128, nch], F32, tag="wv")
                          nc.sync.dma_start(wv_sb[:], WV_d[:, c0 : c0 + nch])
                          # pipelined sub-gathers: prepare_only + trigger on
                          # round-robin SWDGE queues; consumers wait manually
                          # on the queue's DMA-completion sem (tile's auto
                          # waits under-sync multi-prep pipelines).
                          # probe ablations: ehalf gathers 256B elems, edouble 1024B
                          EL = D // 2 if abl == "ehalf" else (2 * D if abl == "edouble" else D)
                          win = src_full[p * W2 : (p + 1) * W2, :]
                          if abl == "ehalf":
                              win = win.rearrange("a (two d) -> (a two) d", two=2)
                          elif abl == "edouble":
                              win = win.rearrange("(a two) d -> a (two d)", two=2)
                          subs = []  # per sub-gather: (tile, a, b, q, tick)
                          if abl != "compute":
                              for a in range(0, nch, GCH):
                                  b = min(a + GCH, nch)
                                  Msub = gpool.tile([128, (b - a) * EL], BF16, tag="gat")
                                  q = qstate["next"]
                                  qstate["next"] = (q + 1) % NQ
                                  qstate["cnt"][q] += 1
                                  gi = nc.gpsimd.dma_gather(
                                      out_ap=Msub[:].rearrange("p (t d) -> p t d", d=EL),
                                      in_ap=win,
                                      idxs_ap=idx_sb[:, a * 8 : b * 8],
                                      num_idxs=(b - a) * 128,
                                      num_idxs_reg=(b - a) * 128,
                                      elem_size=EL,
                                      single_packet=SP,
                                      queue_num=q,
                                  )
                                  if MANUAL_SYNC:
                                      gi.then_inc(qsems[q], 16)
                                  subs.append((Msub, a, b, q, qstate["cnt"][q]))
                          if abl in ("gather", "compute", "ehalf", "edouble"):
                              continue
                          t = 0
                          si = -1
                          Mcur = None
                          for g, ng in gl:
                              if g not in pst:
                                  pst[g] = aps.tile([128, D], F32, tag="agg", name=f"agg{g}")
                              for k in range(ng):
                                  if si < 0 or t >= subs[si][2]:
                                      si += 1
                                      Mcur = subs[si][0]
                                      if MANUAL_SYNC:
                                          nc.tensor.wait_ge(qsems[subs[si][3]], 16 * subs[si][4])
                                  if abl == "nosel":
                                      S = ident
                                  else:
                                      S = spool.tile([128, 128], BF16, tag="sel")
                                      nc.vector.tensor_scalar(
                                          out=S[:], in0=IOTA_sb[:],
                                          scalar1=dest_sb[:, t : t + 1],
                                          scalar2=wv_sb[:, t : t + 1],
                                          op0=AluOp.is_equal, op1=AluOp.mult,
                                      )
                                  nc.tensor.matmul(
                                      out=pst[g][:],
                                      lhsT=S[:],
                                      rhs=Mcur[:, (t - subs[si][1]) * D : (t - subs[si][1] + 1) * D],
                                      start=(ch == first_chunk[g]),
                                      stop=(ch == last_chunk[g]),
                                  )
                                  t += 1
                                  ch += 1
                          # epilogue for any group whose last chunk has been issued
                          done = [g for g in list(pst) if last_chunk[g] < ch]
                          for g in done:
                              h_t = opool.tile([128, D], BF16, tag="h")
                              if not act:
                                  nc.scalar.activation(
                                      out=h_t[:], in_=pst[g][:],
                                      func=mybir.ActivationFunctionType.Copy,
                                      scale=scal[:, g : g + 1],
                                  )
                              elif bias_tile is None:
                                  nc.scalar.activation(
                                      out=h_t[:], in_=pst[g][:],
                                      func=mybir.ActivationFunctionType.Relu,
                                      scale=scal[:, g : g + 1],
                                  )
                              else:
                                  tmp = epool.tile([128, D], F32, tag="tmp")
                                  nc.vector.tensor_scalar(
                                      out=tmp[:], in0=pst[g][:],
                                      scalar1=scal[:, g : g + 1], scalar2=None,
                                      op0=AluOp.mult,
                                  )
                                  nc.vector.tensor_tensor(
                                      out=tmp[:], in0=tmp[:], in1=bias_tile[:], op=AluOp.add
                                  )
                                  nc.vector.tensor_scalar(
                                      out=h_t[:], in0=tmp[:], scalar1=0.0,
                                      scalar2=None, op0=AluOp.max,
                                  )
                              out_cb(g, h_t)
                              del pst[g]

              # conv1: h1 = relu(r * agg + b1)  [r = dinv^2], store h1 to H1BUF
              def out1(g, h):
                  nc.sync.dma_start(H1BUF[g * 128 : (g + 1) * 128, :], h[:])

              if stage >= 3:
                  aggregate(H0F, r_, bias_tiles.get("b1"), out1)

              # ---- AG2: exchange h1' directly (W2 applied post-aggregation)
              if stage >= 5 and collectives:
                  nc.gpsimd.collective_compute(
                      "AllGather", AluOp.bypass, replica_groups=replica,
                      ins=[H1BUF.opt()], outs=[H1F.opt()],
                  )

              # ---- conv2 aggregation + W2 + classifier fused
              with tc.tile_pool(name="w2_ps", bufs=1, space="PSUM") as w2ps, \
                   tc.tile_pool(name="clf_ps", bufs=1, space="PSUM") as clps, \
                   tc.tile_pool(name="tp_ps", bufs=1, space="PSUM") as tpps, \
                   tc.tile_pool(name="h2t", bufs=4) as h2tp, \
                   tc.tile_pool(name="h2s", bufs=2) as h2sp, \
                   tc.tile_pool(name="clf_out", bufs=4) as clop:

                  def out2(g, t_bf):
                      # t_bf = dinv_g * agg2  (bf16, no act). h2 = relu(t @ W2);
                      # out = h2 @ Wc + bc
                      pt = w2ps.tile([128, D], F32, tag="w2o")
                      for b in range(2):
                          tp = tpps.tile([128, 128], BF16, tag="tp")
                          nc.tensor.transpose(
                              out=tp[:], in_=t_bf[:, b * 128 : (b + 1) * 128], identity=ident[:]
                          )
                          tT = h2tp.tile([128, 128], BF16, tag="tT")
                          nc.vector.tensor_copy(tT[:], tp[:])
                          nc.tensor.matmul(
                              out=pt[:],
                              lhsT=tT[:],
                              rhs=W2bf[:, b * D : (b + 1) * D],
                              start=(b == 0),
                              stop=(b == 1),
                          )
                      h2 = h2sp.tile([128, D], BF16, tag="h2")
                      if "b2" in bias_tiles:
                          tmp2 = h2sp.tile([128, D], F32, tag="tmp2")
                          nc.vector.tensor_tensor(
                              out=tmp2[:], in0=pt[:], in1=bias_tiles["b2"][:], op=AluOp.add
                          )
                          nc.vector.tensor_scalar(
                              out=h2[:], in0=tmp2[:], scalar1=0.0, scalar2=None, op0=AluOp.max
                          )
                      else:
                          nc.vector.tensor_scalar(
                              out=h2[:], in0=pt[:], scalar1=0.0, scalar2=None, op0=AluOp.max
                          )
                      cl = clps.tile([128, C], F32, tag="clf")
                      for b in range(2):
                          tp = tpps.tile([128, 128], BF16, tag="tp")
                          nc.tensor.transpose(
                              out=tp[:], in_=h2[:, b * 128 : (b + 1) * 128], identity=ident[:]
                          )
                          h2T = h2tp.tile([128, 128], BF16, tag="tT")
                          nc.vector.tensor_copy(h2T[:], tp[:])
                          nc.tensor.matmul(
                              out=cl[:],
                              lhsT=h2T[:],
                              rhs=Wcbf[:, b * C : (b + 1) * C],
                              start=(b == 0),
                              stop=(b == 1),
                          )
                      co = clop.tile([128, C], F32, tag="co")
                      if "bc" in bias_tiles:
                          nc.vector.tensor_tensor(
                              out=co[:], in0=cl[:], in1=bias_tiles["bc"][:], op=AluOp.add
                          )
                      else:
                          nc.vector.tensor_copy(co[:], cl[:])
                      nc.sync.dma_start(out_d[g * 128 : (g + 1) * 128, :], co[:])

                  if stage >= 6:
                      aggregate(H1F, dinv, None, out2, psum_bufs=5, act=False)

    nc.compile()
    return nc


# ----------------------------------------------------------------------------
# Entry point
# ----------------------------------------------------------------------------
_CACHE = {}


def _get_compiled(x, edge_index, edge_weight, b1, b2, bc):
    cores, meta = _preprocess(x, edge_index, edge_weight)
    has_b1 = bool(np.any(b1))
    has_b2 = bool(np.any(b2))
    has_bc = bool(np.any(bc))
    key = repr((meta["G"], meta["MAXD"], meta["TOTCH"], has_b1, has_b2, has_bc, meta["call_meta"]))
    nc = _CACHE.get(key)
    if nc is None:
        nc = _build(meta, has_b1, has_b2, has_bc)
        _CACHE[key] = nc
    return nc, cores, meta


def make_in_maps(inputs, cores, meta):
    x = np.asarray(inputs["x"], np.float32)
    W1 = np.asarray(inputs["W1"], np.float32)
    W2m = np.asarray(inputs["W2"], np.float32)
    Wc = np.asarray(inputs["Wc"], np.float32)
    b1 = np.asarray(inputs["b1"], np.float32)
    b2 = np.asarray(inputs["b2"], np.float32)
    bc = np.asarray(inputs["bc"], np.float32)
    iota = np.broadcast_to(np.arange(128, dtype=np.float32), (128, 128))
    in_maps = []
    for r in range(R):
        m = dict(
            xT=cores[r]["xT"],
            W1=W1,
            W2m=W2m,
            Wc=Wc,
            IDX16=cores[r]["IDX16"],
            DEST=cores[r]["DEST"],
            WV=cores[r]["WV"],
            WELL=cores[r]["WELL"],
            IOTA=np.ascontiguousarray(iota).astype(ml_dtypes.bfloat16),
            IOTAC=np.arange(128, dtype=np.float32).reshape(128, 1),
        )
        if np.any(b1):
            m["b1b"] = np.ascontiguousarray(np.broadcast_to(b1, (128, D))).astype(np.float32)
        if np.any(b2):
            m["b2b"] = np.ascontiguousarray(np.broadcast_to(b2, (128, D))).astype(np.float32)
        if np.any(bc):
            m["bcb"] = np.ascontiguousarray(np.broadcast_to(bc, (128, C))).astype(np.float32)
        in_maps.append(m)
    return in_maps


def unpack_output(results, meta):
    out = np.empty((N, C), np.float32)
    for r in range(R):
        out[r * NSH : (r + 1) * NSH] = results[r]["out"][meta["pcols"][r]]
    return out


def kernel(**inputs):
    nc, cores, meta = _get_compiled(
        inputs["x"], inputs["edge_index"], inputs["edge_weight"],
        inputs["b1"], inputs["b2"], inputs["bc"],
    )
    in_maps = make_in_maps(inputs, cores, meta)
    res = bass_utils.run_bass_kernel_spmd(nc, in_maps, core_ids=list(range(R)))
    return unpack_output(res.results, meta)



# revision 35
# speedup vs baseline: 1.3837x; 1.0394x over previous
"""GCN (2x GCNConv + linear classifier) on 8 Trainium2 NeuronCores.

Strategy: shard nodes (output rows) across the 8 cores; partition edges by
destination so the scatter-add is core-local; replicate the small weight
matrices; exchange transformed source-node features with an AllGather
before each conv's aggregation.

Aggregation is done on TensorE: edges are grouped by destination into
"groups" of <=128 destinations / <=4096 edges.  For each 128-edge chunk we
build a one-hot selector S[e, d] = w[e] * (dest_local[e] == d) on VectorE
and accumulate S.T @ gathered_messages into the group's PSUM tile.  The
D^-1/2 normalizations are folded into the gather source (dinv * h) and the
epilogue (one fused tensor_scalar: scale + relu).

All heavy data moves in bf16; accumulation in f32 PSUM.
"""

import math
import os
import sys

import numpy as np

sys.path.insert(0, "/opt/trn_rl_repo")

import ml_dtypes  # noqa: E402

from concourse import bass, mybir, bacc, tile  # noqa: E402
from concourse import bass_utils  # noqa: E402

R = 8  # cores
N = 100000
F = 512
D = 256
C = 10
NSH = N // R
GROUP_EDGES = 4096  # 32 chunks of 128
BLK = 4  # dest-groups per gather call block
NQ = 4  # SWDGE queues for pipelined gathers
GCH = 8  # chunks per gather call
SP = False  # single_packet for dma_gather
MANUAL_SYNC = False  # manual qsems/wait_ge vs tile auto-sync for gathers
BF16 = mybir.dt.bfloat16
F32 = mybir.dt.float32
I16 = mybir.dt.int16


def _cdiv(a, b):
    return (a + b - 1) // b


# ----------------------------------------------------------------------------
# Host-side preprocessing: sharding, dest-group packing, gather index tables.
# Pure index manipulation / permutation of the inputs (no float arithmetic).
# ----------------------------------------------------------------------------
def _preprocess(x, edge_index, edge_weight):
    src = np.asarray(edge_index[0], dtype=np.int64)
    dst = np.asarray(edge_index[1], dtype=np.int64)
    w = np.asarray(edge_weight, dtype=np.float32)
    loop = np.arange(N, dtype=np.int64)
    src = np.concatenate([src, loop])
    dst = np.concatenate([dst, loop])
    w = np.concatenate([w, np.ones(N, np.float32)])

    cnt = np.bincount(dst, minlength=N)  # edges into each dest (incl. self)

    # --- per-core greedy dest-group packing (<=128 dests, <=GROUP_EDGES edges)
    core_groups = []  # per core: list of (start_local, end_local)
    for r in range(R):
        c = cnt[r * NSH : (r + 1) * NSH]
        gs = []
        start = 0
        tot = 0
        for i in range(NSH):
            if tot + c[i] > GROUP_EDGES or i - start == 128:
                gs.append((start, i))
                start = i
                tot = 0
            tot += c[i]
        gs.append((start, NSH))
        core_groups.append(gs)

    G = max(len(gs) for gs in core_groups)
    G = _cdiv(G, BLK) * BLK  # pad to block multiple
    assert G <= 127, f"G={G} too large for int16 gather windows"
    GP = G * 128
    W2 = 2 * GP  # rows per gather source window (2 shards)
    assert W2 - 1 <= 32767

    # --- padded numbering
    pcols = []  # per core: local node -> padded column (g*128 + slot)
    grp_ids = []
    for r in range(R):
        gs = core_groups[r]
        pcol = np.empty(NSH, np.int64)
        gid = np.empty(NSH, np.int64)
        for g, (a, b) in enumerate(gs):
            pcol[a:b] = g * 128 + np.arange(b - a)
            gid[a:b] = g
        pcols.append(pcol)
        grp_ids.append(gid)
    prow_global = np.concatenate([r * GP + pcols[r] for r in range(R)])

    # --- per-core edge cell assignment
    ecore = dst // NSH
    per_core = []
    cell_counts = np.zeros((R, G, 4), np.int64)
    for r in range(R):
        m = ecore == r
        es, ed, ew = src[m], dst[m], w[m]
        ld = ed - r * NSH
        g_of = grp_ids[r][ld]
        dl = pcols[r][ld] - g_of * 128
        psrc = prow_global[es]
        p_of = psrc // W2
        cell_counts[r] += np.zeros((G, 4), np.int64)
        np.add.at(cell_counts[r], (g_of, p_of), 1)
        per_core.append((es, ew, g_of, dl, psrc, p_of))

    n_chunks = _cdiv(cell_counts.max(axis=0), 128)  # [G, 4] uniform
    # ensure each group has at least one chunk (PSUM init)
    empty = n_chunks.sum(axis=1) == 0
    n_chunks[empty, 0] = 1

    # processing order: block-major, window-major, group-minor
    order_cells = []  # (g, p) in processing order
    call_meta = []  # per call: (p, [(g, ng)...], chunk_start)
    ch = 0
    for b0 in range(0, G, BLK):
        for p in range(4):
            gl = [(g, int(n_chunks[g, p])) for g in range(b0, b0 + BLK) if n_chunks[g, p] > 0]
            if not gl:
                continue
            call_meta.append((p, gl, ch))
            for g, ng in gl:
                order_cells.append((g, p))
                ch += ng
    TOTCH = ch

    # per-group first/last chunk rank (for PSUM start/stop flags)
    first_chunk = {}
    last_chunk = {}
    ch = 0
    for p, gl, c0 in call_meta:
        for g, ng in gl:
            if g not in first_chunk:
                first_chunk[g] = ch
            last_chunk[g] = ch + ng - 1
            ch += ng

    cell_rank = -np.ones((G, 4), np.int64)
    cell_slot_start = np.zeros(len(order_cells) + 1, np.int64)
    for i, (g, p) in enumerate(order_cells):
        cell_rank[g, p] = i
        cell_slot_start[i + 1] = cell_slot_start[i] + n_chunks[g, p] * 128

    MAXD = int(cnt.max())

    # --- per-core device arrays
    cores = []
    for r in range(R):
        es, ew, g_of, dl, psrc, p_of = per_core[r]
        key = cell_rank[g_of, p_of]
        assert (key >= 0).all()
        order = np.argsort(key, kind="stable")
        ko = key[order]
        # position within cell
        cell_edge_start = np.zeros(len(order_cells), np.int64)
        cc = np.bincount(ko, minlength=len(order_cells))
        cell_edge_start[1:] = np.cumsum(cc)[:-1]
        pos = cell_slot_start[ko] + (np.arange(len(ko)) - cell_edge_start[ko])

        slots_idx = np.zeros(TOTCH * 128, np.int16)
        slots_w = np.zeros(TOTCH * 128, np.float32)
        slots_dl = np.zeros(TOTCH * 128, np.float32)
        slots_idx[pos] = (psrc[order] % W2).astype(np.int16)
        slots_w[pos] = ew[order]
        slots_dl[pos] = dl[order].astype(np.float32)

        # idx16 wrapped per call, concatenated: [128, TOTCH*8]
        idx_parts = []
        for p, gl, c0 in call_meta:
            nch = sum(ng for _, ng in gl)
            fl = slots_idx[c0 * 128 : (c0 + nch) * 128]
            wrapped = fl.reshape(-1, 16).T  # [16, nch*8]
            idx_parts.append(np.tile(wrapped, (8, 1)))
        IDX16 = np.ascontiguousarray(np.concatenate(idx_parts, axis=1))

        DEST = np.ascontiguousarray(slots_dl.reshape(TOTCH, 128).T)
        WV = np.ascontiguousarray(slots_w.reshape(TOTCH, 128).T)

        # ELL weights for degree: [128, G*MAXD]
        pc_e = pcols[r][dst[ecore == r] - r * NSH]
        o2 = np.argsort(pc_e, kind="stable")
        pcs = pc_e[o2]
        dstart = np.zeros(GP, np.int64)
        dcnt = np.bincount(pcs, minlength=GP)
        dstart[1:] = np.cumsum(dcnt)[:-1]
        kidx = np.arange(len(pcs)) - dstart[pcs]
        WELLmat = np.zeros((GP, MAXD), np.float32)
        WELLmat[pcs, kidx] = ew[o2]
        unused = dcnt == 0
        WELLmat[unused, 0] = 1.0  # pad dests get deg=1 (avoid inf)
        WELL = np.ascontiguousarray(
            WELLmat.reshape(G, 128, MAXD).transpose(1, 0, 2).reshape(128, G * MAXD)
        )

        # padded transposed x
        xs = np.asarray(x[r * NSH : (r + 1) * NSH], np.float32)
        xT = np.zeros((F, GP), ml_dtypes.bfloat16)
        xT[:, pcols[r]] = xs.T.astype(ml_dtypes.bfloat16)
        cores.append(dict(IDX16=IDX16, DEST=DEST, WV=WV, WELL=WELL, xT=xT))

    meta = dict(
        G=G,
        GP=GP,
        W2=W2,
        MAXD=MAXD,
        TOTCH=TOTCH,
        call_meta=call_meta,
        first_chunk=first_chunk,
        last_chunk=last_chunk,
        pcols=pcols,
    )
    return cores, meta


# ----------------------------------------------------------------------------
# Device program
# ----------------------------------------------------------------------------
def _build(meta, has_b1, has_b2, has_bc, stage=99, reps=1, abl=None, collectives=True,
           shared_src=True):
    G, GP, W2, MAXD, TOTCH = (
        meta["G"],
        meta["GP"],
        meta["W2"],
        meta["MAXD"],
        meta["TOTCH"],
    )
    call_meta = meta["call_meta"]
    first_chunk, last_chunk = meta["first_chunk"], meta["last_chunk"]

    nc = bacc.Bacc(
        "TRN2",
        target_bir_lowering=False,
        debug=False,
        enable_asserts=False,
        num_devices=R,
        num_swdge_queues=NQ,
        dynamic_dma_scratch_size=32768,
    )
    dt_in = lambda name, shape, dt: nc.dram_tensor(name, shape, dt, kind="ExternalInput").ap()
    xT_d = dt_in("xT", [F, GP], BF16)
    W1_d = dt_in("W1", [F, D], F32)
    W2_d = dt_in("W2m", [D, D], F32)
    Wc_d = dt_in("Wc", [D, C], F32)
    b1_d = dt_in("b1b", [128, D], F32) if has_b1 else None
    b2_d = dt_in("b2b", [128, D], F32) if has_b2 else None
    bc_d = dt_in("bcb", [128, C], F32) if has_bc else None
    IDX_d = dt_in("IDX16", [128, TOTCH * 8], I16)
    DEST_d = dt_in("DEST", [128, TOTCH], F32)
    WV_d = dt_in("WV", [128, TOTCH], F32)
    WELL_d = dt_in("WELL", [128, G * MAXD], F32)
    IOTA_d = dt_in("IOTA", [128, 128], BF16)
    IOTAC_d = dt_in("IOTAC", [128, 1], F32)
    out_d = nc.dram_tensor("out", [GP, C], F32, kind="ExternalOutput").ap()

    AluOp = mybir.AluOpType
    replica = [list(range(R))]

    qsems = [nc.alloc_semaphore(f"gq{q}") for q in range(NQ)]
    qstate = {"next": 0, "cnt": [0] * NQ}

    with tile.TileContext(nc) as tc:
        with tc.tile_pool(name="const", bufs=1) as cpool, \
             tc.tile_pool(name="dram", bufs=1, space="DRAM") as dr:
            # ---- resident constants
            W1bf = cpool.tile([128, 4 * D], BF16)
            for b in range(4):
                nc.gpsimd.dma_start(W1bf[:, b * D : (b + 1) * D], W1_d[b * 128 : (b + 1) * 128, :])
            W2bf = cpool.tile([128, 2 * D], BF16)
            for b in range(2):
                nc.gpsimd.dma_start(W2bf[:, b * D : (b + 1) * D], W2_d[b * 128 : (b + 1) * 128, :])
            Wcbf = cpool.tile([128, 2 * C], BF16)
            for b in range(2):
                nc.gpsimd.dma_start(Wcbf[:, b * C : (b + 1) * C], Wc_d[b * 128 : (b + 1) * 128, :])
            IOTA_sb = cpool.tile([128, 128], BF16)
            nc.sync.dma_start(IOTA_sb[:], IOTA_d[:])
            iotac = cpool.tile([128, 1], F32)
            nc.sync.dma_start(iotac[:], IOTAC_d[:])
            ident = cpool.tile([128, 128], BF16)
            nc.vector.tensor_scalar(
                out=ident[:], in0=IOTA_sb[:], scalar1=iotac[:], scalar2=None,
                op0=AluOp.is_equal,
            )
            bias_tiles = {}
            for nm, d_ap, has in (("b1", b1_d, has_b1), ("b2", b2_d, has_b2), ("bc", bc_d, has_bc)):
                if has:
                    t = cpool.tile([128, d_ap.shape[1]], F32, name=f"bias_{nm}")
                    nc.sync.dma_start(t[:], d_ap[:])
                    bias_tiles[nm] = t

            # ---- degrees -> r = 1/deg, dinv = 1/sqrt(deg)
            deg = cpool.tile([128, G], F32)
            r_ = cpool.tile([128, G], F32)
            dinv = cpool.tile([128, G], F32)
            with tc.tile_pool(name="well", bufs=1) as wpool:
                well_sb = wpool.tile([128, G * MAXD], F32)
                nc.sync.dma_start(well_sb[:], WELL_d[:])
                nc.vector.tensor_reduce(
                    out=deg[:],
                    in_=well_sb[:].rearrange("p (g m) -> p g m", m=MAXD),
                    axis=mybir.AxisListType.X,
                    op=AluOp.add,
                )
            nc.vector.reciprocal(r_[:], deg[:])
            nc.scalar.sqrt(dinv[:], r_[:])

            for _rep in range(reps):
              # ---- DRAM buffers (per rep: Shared tiles allow one writer)
              spc = dict(addr_space="Shared") if shared_src else {}
              AGIN0 = dr.tile([GP, D], BF16, name=f"AGIN0_{_rep}")
              H0F = dr.tile([R * GP, D], BF16, name=f"H0F_{_rep}", **spc)
              H1BUF = dr.tile([GP, D], BF16, name=f"H1BUF_{_rep}")
              H1F = dr.tile([R * GP, D], BF16, name=f"H1F_{_rep}", **spc)
              # ---- stage B: h0 = dinv * (x @ W1), write to AGIN0
              NB = 8
              with tc.tile_pool(name="xg", bufs=2) as xpool, \
                   tc.tile_pool(name="bps", bufs=4, space="PSUM") as bps, \
                   tc.tile_pool(name="hstage", bufs=4) as hpool:
                  for gc in range(0, G, NB):
                      nb = min(NB, G - gc)
                      xg = xpool.tile([128, 4 * nb * 128], BF16, tag="xg")
                      for b in range(4):
                          nc.sync.dma_start(
                              xg[:, b * nb * 128 : (b + 1) * nb * 128],
                              xT_d[b * 128 : (b + 1) * 128, gc * 128 : (gc + nb) * 128],
                          )
                      for gi in range(nb):
                          g = gc + gi
                          pt = bps.tile([128, D], F32, tag="bpt")
                          for b in range(4):
                              nc.tensor.matmul(
                                  out=pt[:],
                                  lhsT=xg[:, (b * nb + gi) * 128 : (b * nb + gi + 1) * 128],
                                  rhs=W1bf[:, b * D : (b + 1) * D],
                                  start=(b == 0),
                                  stop=(b == 3),
                              )
                          h0 = hpool.tile([128, D], BF16, tag="h0")
                          nc.vector.tensor_scalar(
                              out=h0[:], in0=pt[:], scalar1=dinv[:, g : g + 1],
                              scalar2=None, op0=AluOp.mult,
                          )
                          nc.sync.dma_start(AGIN0[g * 128 : (g + 1) * 128, :], h0[:])

              # ---- AG1
              if stage >= 2 and collectives:
                  nc.gpsimd.collective_compute(
                      "AllGather", AluOp.bypass, replica_groups=replica,
                      ins=[AGIN0.opt()], outs=[H0F.opt()],
                  )

              # ---- conv aggregation (shared for conv1 / conv2)
              def aggregate(src_full, scal, bias_tile, out_cb, psum_bufs=8, act=True):
                  """scatter-add aggregation over all groups.

                  src_full: [R*GP, D] gather source; scal: [128,G] per-dest scale
                  applied pre-relu; out_cb(g, tile_bf16) consumes the result.
                  """
                  with tc.tile_pool(name="agg_ps", bufs=psum_bufs, space="PSUM") as aps, \
                       tc.tile_pool(name="gat", bufs=10) as gpool, \
                       tc.tile_pool(name="meta", bufs=4) as mpool, \
                       tc.tile_pool(name="sel", bufs=12) as spool, \
                       tc.tile_pool(name="hout", bufs=4) as opool, \
                       tc.tile_pool(name="epi", bufs=2) as epool:
                      pst = {}
                      ch = 0
                      for ci, (p, gl, c0) in enumerate(call_meta):
                          nch = sum(ng for _, ng in gl)
                          idx_sb = mpool.tile([128, nch * 8], I16, tag="idx")
                          nc.sync.dma_start(idx_sb[:], IDX_d[:, c0 * 8 : (c0 + nch) * 8])
                          dest_sb = mpool.tile([128, nch], F32, tag="dst")
                          nc.sync.dma_start(dest_sb[:], DEST_d[:, c0 : c0 + nch])
                          wv_sb = mpool.tile([128, nch], F32, tag="wv")
                          nc.sync.dma_start(wv_sb[:], WV_d[:, c0 : c0 + nch])
                          # pipelined sub-gathers: prepare_only + trigger on
                          # round-robin SWDGE queues; consumers wait manually
                          # on the queue's DMA-completion sem (tile's auto
                          # waits under-sync multi-prep pipelines).
                          # probe ablations: ehalf gathers 256B elems, edouble 1024B
                          EL = D // 2 if abl == "ehalf" else (2 * D if abl == "edouble" else D)
                          win = src_full[p * W2 : (p + 1) * W2, :]
                          if abl == "ehalf":
                              win = win.rearrange("a (two d) -> (a two) d", two=2)
                          elif abl == "edouble":
                              win = win.rearrange("(a two) d -> a (two d)", two=2)
                          subs = []  # per sub-gather: (tile, a, b, q, tick)
                          if abl != "compute":
                              for a in range(0, nch, GCH):
                                  b = min(a + GCH, nch)
                                  Msub = gpool.tile([128, (b - a) * EL], BF16, tag="gat")
                                  q = qstate["next"]
                                  qstate["next"] = (q + 1) % NQ
                                  qstate["cnt"][q] += 1
                                  gi = nc.gpsimd.dma_gather(
                                      out_ap=Msub[:].rearrange("p (t d) -> p t d", d=EL),
                                      in_ap=win,
                                      idxs_ap=idx_sb[:, a * 8 : b * 8],
                                      num_idxs=(b - a) * 128,
                                      num_idxs_reg=(b - a) * 128,
                                      elem_size=EL,
                                      single_packet=SP,
                                      queue_num=q,
                                  )
                                  if MANUAL_SYNC:
                                      gi.then_inc(qsems[q], 16)
                                  subs.append((Msub, a, b, q, qstate["cnt"][q]))
                          if abl in ("gather", "compute", "ehalf", "edouble"):
                              continue
                          t = 0
                          si = -1
                          Mcur = None
                          for g, ng in gl:
                              if g not in pst:
                                  pst[g] = aps.tile([128, D], F32, tag="agg", name=f"agg{g}")
                              for k in range(ng):
                                  if si < 0 or t >= subs[si][2]:
                                      si += 1
                                      Mcur = subs[si][0]
                                      if MANUAL_SYNC:
                                          nc.tensor.wait_ge(qsems[subs[si][3]], 16 * subs[si][4])
                                  if abl == "nosel":
                                      S = ident
                                  else:
                                      S = spool.tile([128, 128], BF16, tag="sel")
                                      nc.vector.tensor_scalar(
                                          out=S[:], in0=IOTA_sb[:],
                                          scalar1=dest_sb[:, t : t + 1],
                                          scalar2=wv_sb[:, t : t + 1],
                                          op0=AluOp.is_equal, op1=AluOp.mult,
                                      )
                                  nc.tensor.matmul(
                                      out=pst[g][:],
                                      lhsT=S[:],
                                      rhs=Mcur[:, (t - subs[si][1]) * D : (t - subs[si][1] + 1) * D],
                                      start=(ch == first_chunk[g]),
                                      stop=(ch == last_chunk[g]),
                                  )
                                  t += 1
                                  ch += 1
                          # epilogue for any group whose last chunk has been issued
                          done = [g for g in list(pst) if last_chunk[g] < ch]
                          for g in done:
                              h_t = opool.tile([128, D], BF16, tag="h")
                              if not act:
                                  nc.vector.tensor_scalar(
                                      out=h_t[:], in0=pst[g][:],
                                      scalar1=scal[:, g : g + 1], scalar2=None,
                                      op0=AluOp.mult,
                                  )
                              elif bias_tile is None:
                                  nc.vector.tensor_scalar(
                                      out=h_t[:], in0=pst[g][:],
                                      scalar1=scal[:, g : g + 1], scalar2=0.0,
                                      op0=AluOp.mult, op1=AluOp.max,
                                  )
                              else:
                                  tmp = epool.tile([128, D], F32, tag="tmp")
                                  nc.vector.tensor_scalar(
                                      out=tmp[:], in0=pst[g][:],
                                      scalar1=scal[:, g : g + 1], scalar2=None,
                                      op0=AluOp.mult,
                                  )
                                  nc.vector.tensor_tensor(
                                      out=tmp[:], in0=tmp[:], in1=bias_tile[:], op=AluOp.add
                                  )
                                  nc.vector.tensor_scalar(
                                      out=h_t[:], in0=tmp[:], scalar1=0.0,
                                      scalar2=None, op0=AluOp.max,
                                  )
                              out_cb(g, h_t)
                              del pst[g]

              # conv1: h1 = relu(r * agg + b1)  [r = dinv^2], store h1 to H1BUF
              def out1(g, h):
                  nc.sync.dma_start(H1BUF[g * 128 : (g + 1) * 128, :], h[:])

              if stage >= 3:
                  aggregate(H0F, r_, bias_tiles.get("b1"), out1)

              # ---- AG2: exchange h1' directly (W2 applied post-aggregation)
              if stage >= 5 and collectives:
                  nc.gpsimd.collective_compute(
                      "AllGather", AluOp.bypass, replica_groups=replica,
                      ins=[H1BUF.opt()], outs=[H1F.opt()],
                  )

              # ---- conv2 aggregation + W2 + classifier fused
              with tc.tile_pool(name="w2_ps", bufs=1, space="PSUM") as w2ps, \
                   tc.tile_pool(name="clf_ps", bufs=1, space="PSUM") as clps, \
                   tc.tile_pool(name="tp_ps", bufs=1, space="PSUM") as tpps, \
                   tc.tile_pool(name="h2t", bufs=4) as h2tp, \
                   tc.tile_pool(name="h2s", bufs=2) as h2sp, \
                   tc.tile_pool(name="clf_out", bufs=4) as clop:

                  def out2(g, t_bf):
                      # t_bf = dinv_g * agg2  (bf16, no act). h2 = relu(t @ W2);
                      # out = h2 @ Wc + bc
                      pt = w2ps.tile([128, D], F32, tag="w2o")
                      for b in range(2):
                          tp = tpps.tile([128, 128], BF16, tag="tp")
                          nc.tensor.transpose(
                              out=tp[:], in_=t_bf[:, b * 128 : (b + 1) * 128], identity=ident[:]
                          )
                          tT = h2tp.tile([128, 128], BF16, tag="tT")
                          nc.vector.tensor_copy(tT[:], tp[:])
                          nc.tensor.matmul(
                              out=pt[:],
                              lhsT=tT[:],
                              rhs=W2bf[:, b * D : (b + 1) * D],
                              start=(b == 0),
                              stop=(b == 1),
                          )
                      h2 = h2sp.tile([128, D], BF16, tag="h2")
                      if "b2" in bias_tiles:
                          tmp2 = h2sp.tile([128, D], F32, tag="tmp2")
                          nc.vector.tensor_tensor(
                              out=tmp2[:], in0=pt[:], in1=bias_tiles["b2"][:], op=AluOp.add
                          )
                          nc.vector.tensor_scalar(
                              out=h2[:], in0=tmp2[:], scalar1=0.0, scalar2=None, op0=AluOp.max
                          )
                      else:
                          nc.vector.tensor_scalar(
                              out=h2[:], in0=pt[:], scalar1=0.0, scalar2=None, op0=AluOp.max
                          )
                      cl = clps.tile([128, C], F32, tag="clf")
                      for b in range(2):
                          tp = tpps.tile([128, 128], BF16, tag="tp")
                          nc.tensor.transpose(
                              out=tp[:], in_=h2[:, b * 128 : (b + 1) * 128], identity=ident[:]
                          )
                          h2T = h2tp.tile([128, 128], BF16, tag="tT")
                          nc.vector.tensor_copy(h2T[:], tp[:])
                          nc.tensor.matmul(
                              out=cl[:],
                              lhsT=h2T[:],
                              rhs=Wcbf[:, b * C : (b + 1) * C],
                              start=(b == 0),
                              stop=(b == 1),
                          )
                      co = clop.tile([128, C], F32, tag="co")
                      if "bc" in bias_tiles:
                          nc.vector.tensor_tensor(
                              out=co[:], in0=cl[:], in1=bias_tiles["bc"][:], op=AluOp.add
                          )
                      else:
                          nc.vector.tensor_copy(co[:], cl[:])
                      nc.sync.dma_start(out_d[g * 128 : (g + 1) * 128, :], co[:])

                  if stage >= 6:
                      aggregate(H1F, dinv, None, out2, psum_bufs=5, act=False)

    nc.compile()
    return nc


# ----------------------------------------------------------------------------
# Entry point
# ----------------------------------------------------------------------------
_CACHE = {}


def _get_compiled(x, edge_index, edge_weight, b1, b2, bc):
    cores, meta = _preprocess(x, edge_index, edge_weight)
    has_b1 = bool(np.any(b1))
    has_b2 = bool(np.any(b2))
    has_bc = bool(np.any(bc))
    key = repr((meta["G"], meta["MAXD"], meta["TOTCH"], has_b1, has_b2, has_bc, meta["call_meta"]))
    nc = _CACHE.get(key)
    if nc is None:
        nc = _build(meta, has_b1, has_b2, has_bc)
        _CACHE[key] = nc
    return nc, cores, meta


def make_in_maps(inputs, cores, meta):
    x = np.asarray(inputs["x"], np.float32)
    W1 = np.asarray(inputs["W1"], np.float32)
    W2m = np.asarray(inputs["W2"], np.float32)
    Wc = np.asarray(inputs["Wc"], np.float32)
    b1 = np.asarray(inputs["b1"], np.float32)
    b2 = np.asarray(inputs["b2"], np.float32)
    bc = np.asarray(inputs["bc"], np.float32)
    iota = np.broadcast_to(np.arange(128, dtype=np.float32), (128, 128))
    in_maps = []
    for r in range(R):
        m = dict(
            xT=cores[r]["xT"],
            W1=W1,
            W2m=W2m,
            Wc=Wc,
            IDX16=cores[r]["IDX16"],
            DEST=cores[r]["DEST"],
            WV=cores[r]["WV"],
            WELL=cores[r]["WELL"],
            IOTA=np.ascontiguousarray(iota).astype(ml_dtypes.bfloat16),
            IOTAC=np.arange(128, dtype=np.float32).reshape(128, 1),
        )
        if np.any(b1):
            m["b1b"] = np.ascontiguousarray(np.broadcast_to(b1, (128, D))).astype(np.float32)
        if np.any(b2):
            m["b2b"] = np.ascontiguousarray(np.broadcast_to(b2, (128, D))).astype(np.float32)
        if np.any(bc):
            m["bcb"] = np.ascontiguousarray(np.broadcast_to(bc, (128, C))).astype(np.float32)
        in_maps.append(m)
    return in_maps


def unpack_output(results, meta):
    out = np.empty((N, C), np.float32)
    for r in range(R):
        out[r * NSH : (r + 1) * NSH] = results[r]["out"][meta["pcols"][r]]
    return out


def kernel(**inputs):
    nc, cores, meta = _get_compiled(
        inputs["x"], inputs["edge_index"], inputs["edge_weight"],
        inputs["b1"], inputs["b2"], inputs["bc"],
    )
    in_maps = make_in_maps(inputs, cores, meta)
    res = bass_utils.run_bass_kernel_spmd(nc, in_maps, core_ids=list(range(R)))
    return unpack_output(res.results, meta)



# revision 40
# speedup vs baseline: 1.7065x; 1.2333x over previous
"""GCN (2x GCNConv + linear classifier) on 8 Trainium2 NeuronCores.

Strategy: shard nodes (output rows) across the 8 cores; partition edges by
destination so the scatter-add is core-local; replicate the small weight
matrices; exchange transformed source-node features with an AllGather
before each conv's aggregation.

Aggregation is done on TensorE: edges are grouped by destination into
"groups" of <=128 destinations / <=4096 edges.  For each 128-edge chunk we
build a one-hot selector S[e, d] = w[e] * (dest_local[e] == d) on VectorE
and accumulate S.T @ gathered_messages into the group's PSUM tile.  The
D^-1/2 normalizations are folded into the gather source (dinv * h) and the
epilogue (one fused tensor_scalar: scale + relu).

All heavy data moves in bf16; accumulation in f32 PSUM.
"""

import math
import os
import sys

import numpy as np

sys.path.insert(0, "/opt/trn_rl_repo")

import ml_dtypes  # noqa: E402

from concourse import bass, mybir, bacc, tile  # noqa: E402
from concourse import bass_utils  # noqa: E402

R = 8  # cores
N = 100000
F = 512
D = 256
C = 10
NSH = N // R
GROUP_EDGES = 4096  # 32 chunks of 128
BLK = 4  # dest-groups per gather call block
NQ = 4  # SWDGE queues for pipelined gathers
GCH = 8  # chunks per gather call
SP = False  # single_packet for dma_gather
MANUAL_SYNC = False  # manual qsems/wait_ge vs tile auto-sync for gathers
BF16 = mybir.dt.bfloat16
F32 = mybir.dt.float32
I16 = mybir.dt.int16


def _cdiv(a, b):
    return (a + b - 1) // b


# ----------------------------------------------------------------------------
# Host-side preprocessing: sharding, dest-group packing, gather index tables.
# Pure index manipulation / permutation of the inputs (no float arithmetic).
# ----------------------------------------------------------------------------
def _preprocess(x, edge_index, edge_weight):
    src = np.asarray(edge_index[0], dtype=np.int64)
    dst = np.asarray(edge_index[1], dtype=np.int64)
    w = np.asarray(edge_weight, dtype=np.float32)
    loop = np.arange(N, dtype=np.int64)
    src = np.concatenate([src, loop])
    dst = np.concatenate([dst, loop])
    w = np.concatenate([w, np.ones(N, np.float32)])

    cnt = np.bincount(dst, minlength=N)  # edges into each dest (incl. self)

    # --- per-core greedy dest-group packing (<=128 dests, <=GROUP_EDGES edges)
    core_groups = []  # per core: list of (start_local, end_local)
    for r in range(R):
        c = cnt[r * NSH : (r + 1) * NSH]
        gs = []
        start = 0
        tot = 0
        for i in range(NSH):
            if tot + c[i] > GROUP_EDGES or i - start == 128:
                gs.append((start, i))
                start = i
                tot = 0
            tot += c[i]
        gs.append((start, NSH))
        core_groups.append(gs)

    G = max(len(gs) for gs in core_groups)
    G = _cdiv(G, BLK) * BLK  # pad to block multiple
    assert G <= 127, f"G={G} too large for int16 gather windows"
    GP = G * 128
    W2 = 2 * GP  # rows per gather source window (2 shards)
    assert W2 - 1 <= 32767

    # --- padded numbering
    pcols = []  # per core: local node -> padded column (g*128 + slot)
    grp_ids = []
    for r in range(R):
        gs = core_groups[r]
        pcol = np.empty(NSH, np.int64)
        gid = np.empty(NSH, np.int64)
        for g, (a, b) in enumerate(gs):
            pcol[a:b] = g * 128 + np.arange(b - a)
            gid[a:b] = g
        pcols.append(pcol)
        grp_ids.append(gid)
    prow_global = np.concatenate([r * GP + pcols[r] for r in range(R)])

    # --- per-core edge cell assignment
    ecore = dst // NSH
    per_core = []
    cell_counts = np.zeros((R, G, 4), np.int64)
    for r in range(R):
        m = ecore == r
        es, ed, ew = src[m], dst[m], w[m]
        ld = ed - r * NSH
        g_of = grp_ids[r][ld]
        dl = pcols[r][ld] - g_of * 128
        psrc = prow_global[es]
        p_of = psrc // W2
        cell_counts[r] += np.zeros((G, 4), np.int64)
        np.add.at(cell_counts[r], (g_of, p_of), 1)
        per_core.append((es, ew, g_of, dl, psrc, p_of))

    n_chunks = _cdiv(cell_counts.max(axis=0), 128)  # [G, 4] uniform
    # ensure each group has at least one chunk (PSUM init)
    empty = n_chunks.sum(axis=1) == 0
    n_chunks[empty, 0] = 1

    # processing order: block-major, window-major, group-minor
    order_cells = []  # (g, p) in processing order
    call_meta = []  # per call: (p, [(g, ng)...], chunk_start)
    ch = 0
    for b0 in range(0, G, BLK):
        for p in range(4):
            gl = [(g, int(n_chunks[g, p])) for g in range(b0, b0 + BLK) if n_chunks[g, p] > 0]
            if not gl:
                continue
            call_meta.append((p, gl, ch))
            for g, ng in gl:
                order_cells.append((g, p))
                ch += ng
    TOTCH = ch

    # per-group first/last chunk rank (for PSUM start/stop flags)
    first_chunk = {}
    last_chunk = {}
    ch = 0
    for p, gl, c0 in call_meta:
        for g, ng in gl:
            if g not in first_chunk:
                first_chunk[g] = ch
            last_chunk[g] = ch + ng - 1
            ch += ng

    cell_rank = -np.ones((G, 4), np.int64)
    cell_slot_start = np.zeros(len(order_cells) + 1, np.int64)
    for i, (g, p) in enumerate(order_cells):
        cell_rank[g, p] = i
        cell_slot_start[i + 1] = cell_slot_start[i] + n_chunks[g, p] * 128

    MAXD = int(cnt.max())

    # --- per-core device arrays
    cores = []
    for r in range(R):
        es, ew, g_of, dl, psrc, p_of = per_core[r]
        key = cell_rank[g_of, p_of]
        assert (key >= 0).all()
        order = np.argsort(key, kind="stable")
        ko = key[order]
        # position within cell
        cell_edge_start = np.zeros(len(order_cells), np.int64)
        cc = np.bincount(ko, minlength=len(order_cells))
        cell_edge_start[1:] = np.cumsum(cc)[:-1]
        pos = cell_slot_start[ko] + (np.arange(len(ko)) - cell_edge_start[ko])

        slots_idx = np.zeros(TOTCH * 128, np.int16)
        slots_w = np.zeros(TOTCH * 128, np.float32)
        slots_dl = np.zeros(TOTCH * 128, np.float32)
        slots_idx[pos] = (psrc[order] % W2).astype(np.int16)
        slots_w[pos] = ew[order]
        slots_dl[pos] = dl[order].astype(np.float32)

        # idx16 wrapped per call, concatenated: [128, TOTCH*8]
        idx_parts = []
        for p, gl, c0 in call_meta:
            nch = sum(ng for _, ng in gl)
            fl = slots_idx[c0 * 128 : (c0 + nch) * 128]
            wrapped = fl.reshape(-1, 16).T  # [16, nch*8]
            idx_parts.append(np.tile(wrapped, (8, 1)))
        IDX16 = np.ascontiguousarray(np.concatenate(idx_parts, axis=1))

        DEST = np.ascontiguousarray(slots_dl.reshape(TOTCH, 128).T)
        WV = np.ascontiguousarray(slots_w.reshape(TOTCH, 128).T)

        # ELL weights for degree: [128, G*MAXD]
        pc_e = pcols[r][dst[ecore == r] - r * NSH]
        o2 = np.argsort(pc_e, kind="stable")
        pcs = pc_e[o2]
        dstart = np.zeros(GP, np.int64)
        dcnt = np.bincount(pcs, minlength=GP)
        dstart[1:] = np.cumsum(dcnt)[:-1]
        kidx = np.arange(len(pcs)) - dstart[pcs]
        WELLmat = np.zeros((GP, MAXD), np.float32)
        WELLmat[pcs, kidx] = ew[o2]
        unused = dcnt == 0
        WELLmat[unused, 0] = 1.0  # pad dests get deg=1 (avoid inf)
        WELL = np.ascontiguousarray(
            WELLmat.reshape(G, 128, MAXD).transpose(1, 0, 2).reshape(128, G * MAXD)
        )

        # padded transposed x
        xs = np.asarray(x[r * NSH : (r + 1) * NSH], np.float32)
        xT = np.zeros((F, GP), ml_dtypes.bfloat16)
        xT[:, pcols[r]] = xs.T.astype(ml_dtypes.bfloat16)
        cores.append(dict(IDX16=IDX16, DEST=DEST, WV=WV, WELL=WELL, xT=xT))

    meta = dict(
        G=G,
        GP=GP,
        W2=W2,
        MAXD=MAXD,
        TOTCH=TOTCH,
        call_meta=call_meta,
        first_chunk=first_chunk,
        last_chunk=last_chunk,
        pcols=pcols,
    )
    return cores, meta


# ----------------------------------------------------------------------------
# Device program
# ----------------------------------------------------------------------------
def _build(meta, has_b1, has_b2, has_bc, stage=99, reps=1, abl=None, collectives=True,
           shared_src=True):
    G, GP, W2, MAXD, TOTCH = (
        meta["G"],
        meta["GP"],
        meta["W2"],
        meta["MAXD"],
        meta["TOTCH"],
    )
    call_meta = meta["call_meta"]
    first_chunk, last_chunk = meta["first_chunk"], meta["last_chunk"]

    nc = bacc.Bacc(
        "TRN2",
        target_bir_lowering=False,
        debug=False,
        enable_asserts=False,
        num_devices=R,
        num_swdge_queues=NQ,
        dynamic_dma_scratch_size=32768,
    )
    dt_in = lambda name, shape, dt: nc.dram_tensor(name, shape, dt, kind="ExternalInput").ap()
    xT_d = dt_in("xT", [F, GP], BF16)
    W1_d = dt_in("W1", [F, D], F32)
    W2_d = dt_in("W2m", [D, D], F32)
    Wc_d = dt_in("Wc", [D, C], F32)
    b1_d = dt_in("b1b", [128, D], F32) if has_b1 else None
    b2_d = dt_in("b2b", [128, D], F32) if has_b2 else None
    bc_d = dt_in("bcb", [128, C], F32) if has_bc else None
    IDX_d = dt_in("IDX16", [128, TOTCH * 8], I16)
    SBIG_d = dt_in("SBIG", [128, TOTCH * 128], BF16)
    WELL_d = dt_in("WELL", [128, G * MAXD], F32)
    IOTA_d = dt_in("IOTA", [128, 128], BF16)
    IOTAC_d = dt_in("IOTAC", [128, 1], F32)
    out_d = nc.dram_tensor("out", [GP, C], F32, kind="ExternalOutput").ap()

    AluOp = mybir.AluOpType
    replica = [list(range(R))]

    qsems = [nc.alloc_semaphore(f"gq{q}") for q in range(NQ)]
    qstate = {"next": 0, "cnt": [0] * NQ}

    with tile.TileContext(nc) as tc:
        with tc.tile_pool(name="const", bufs=1) as cpool, \
             tc.tile_pool(name="dram", bufs=1, space="DRAM") as dr:
            # ---- resident constants
            W1bf = cpool.tile([128, 4 * D], BF16)
            for b in range(4):
                nc.gpsimd.dma_start(W1bf[:, b * D : (b + 1) * D], W1_d[b * 128 : (b + 1) * 128, :])
            W2bf = cpool.tile([128, 2 * D], BF16)
            for b in range(2):
                nc.gpsimd.dma_start(W2bf[:, b * D : (b + 1) * D], W2_d[b * 128 : (b + 1) * 128, :])
            Wcbf = cpool.tile([128, 2 * C], BF16)
            for b in range(2):
                nc.gpsimd.dma_start(Wcbf[:, b * C : (b + 1) * C], Wc_d[b * 128 : (b + 1) * 128, :])
            IOTA_sb = cpool.tile([128, 128], BF16)
            nc.sync.dma_start(IOTA_sb[:], IOTA_d[:])
            iotac = cpool.tile([128, 1], F32)
            nc.sync.dma_start(iotac[:], IOTAC_d[:])
            ident = cpool.tile([128, 128], BF16)
            nc.vector.tensor_scalar(
                out=ident[:], in0=IOTA_sb[:], scalar1=iotac[:], scalar2=None,
                op0=AluOp.is_equal,
            )
            bias_tiles = {}
            for nm, d_ap, has in (("b1", b1_d, has_b1), ("b2", b2_d, has_b2), ("bc", bc_d, has_bc)):
                if has:
                    t = cpool.tile([128, d_ap.shape[1]], F32, name=f"bias_{nm}")
                    nc.sync.dma_start(t[:], d_ap[:])
                    bias_tiles[nm] = t

            # ---- degrees -> r = 1/deg, dinv = 1/sqrt(deg)
            deg = cpool.tile([128, G], F32)
            r_ = cpool.tile([128, G], F32)
            dinv = cpool.tile([128, G], F32)
            with tc.tile_pool(name="well", bufs=1) as wpool:
                well_sb = wpool.tile([128, G * MAXD], F32)
                nc.sync.dma_start(well_sb[:], WELL_d[:])
                nc.vector.tensor_reduce(
                    out=deg[:],
                    in_=well_sb[:].rearrange("p (g m) -> p g m", m=MAXD),
                    axis=mybir.AxisListType.X,
                    op=AluOp.add,
                )
            nc.vector.reciprocal(r_[:], deg[:])
            nc.scalar.sqrt(dinv[:], r_[:])

            for _rep in range(reps):
              # ---- DRAM buffers (per rep: Shared tiles allow one writer)
              spc = dict(addr_space="Shared") if shared_src else {}
              AGIN0 = dr.tile([GP, D], BF16, name=f"AGIN0_{_rep}")
              H0F = dr.tile([R * GP, D], BF16, name=f"H0F_{_rep}", **spc)
              H1BUF = dr.tile([GP, D], BF16, name=f"H1BUF_{_rep}")
              H1F = dr.tile([R * GP, D], BF16, name=f"H1F_{_rep}", **spc)
              # ---- stage B: h0 = dinv * (x @ W1), write to AGIN0
              NB = 8
              with tc.tile_pool(name="xg", bufs=2) as xpool, \
                   tc.tile_pool(name="bps", bufs=4, space="PSUM") as bps, \
                   tc.tile_pool(name="hstage", bufs=4) as hpool:
                  for gc in range(0, G, NB):
                      nb = min(NB, G - gc)
                      xg = xpool.tile([128, 4 * nb * 128], BF16, tag="xg")
                      for b in range(4):
                          nc.sync.dma_start(
                              xg[:, b * nb * 128 : (b + 1) * nb * 128],
                              xT_d[b * 128 : (b + 1) * 128, gc * 128 : (gc + nb) * 128],
                          )
                      for gi in range(nb):
                          g = gc + gi
                          pt = bps.tile([128, D], F32, tag="bpt")
                          for b in range(4):
                              nc.tensor.matmul(
                                  out=pt[:],
                                  lhsT=xg[:, (b * nb + gi) * 128 : (b * nb + gi + 1) * 128],
                                  rhs=W1bf[:, b * D : (b + 1) * D],
                                  start=(b == 0),
                                  stop=(b == 3),
                              )
                          h0 = hpool.tile([128, D], BF16, tag="h0")
                          nc.vector.tensor_scalar(
                              out=h0[:], in0=pt[:], scalar1=dinv[:, g : g + 1],
                              scalar2=None, op0=AluOp.mult,
                          )
                          nc.sync.dma_start(AGIN0[g * 128 : (g + 1) * 128, :], h0[:])

              # ---- AG1
              if stage >= 2 and collectives:
                  nc.gpsimd.collective_compute(
                      "AllGather", AluOp.bypass, replica_groups=replica,
                      ins=[AGIN0.opt()], outs=[H0F.opt()],
                  )

              # ---- conv aggregation (shared for conv1 / conv2)
              def aggregate(src_full, scal, bias_tile, out_cb, psum_bufs=8, act=True):
                  """scatter-add aggregation over all groups.

                  src_full: [R*GP, D] gather source; scal: [128,G] per-dest scale
                  applied pre-relu; out_cb(g, tile_bf16) consumes the result.
                  """
                  with tc.tile_pool(name="agg_ps", bufs=psum_bufs, space="PSUM") as aps, \
                       tc.tile_pool(name="gat", bufs=10) as gpool, \
                       tc.tile_pool(name="meta", bufs=4) as mpool, \
                       tc.tile_pool(name="sel", bufs=12) as spool, \
                       tc.tile_pool(name="hout", bufs=4) as opool, \
                       tc.tile_pool(name="epi", bufs=2) as epool:
                      pst = {}
                      ch = 0
                      for ci, (p, gl, c0) in enumerate(call_meta):
                          nch = sum(ng for _, ng in gl)
                          idx_sb = mpool.tile([128, nch * 8], I16, tag="idx")
                          nc.sync.dma_start(idx_sb[:], IDX_d[:, c0 * 8 : (c0 + nch) * 8])
                          sbig_sb = mpool.tile([128, nch * 128], BF16, tag="sbig")
                          nc.sync.dma_start(sbig_sb[:], SBIG_d[:, c0 * 128 : (c0 + nch) * 128])
                          # pipelined sub-gathers: prepare_only + trigger on
                          # round-robin SWDGE queues; consumers wait manually
                          # on the queue's DMA-completion sem (tile's auto
                          # waits under-sync multi-prep pipelines).
                          # probe ablations: ehalf gathers 256B elems, edouble 1024B
                          EL = D // 2 if abl == "ehalf" else (2 * D if abl == "edouble" else D)
                          win = src_full[p * W2 : (p + 1) * W2, :]
                          if abl == "ehalf":
                              win = win.rearrange("a (two d) -> (a two) d", two=2)
                          elif abl == "edouble":
                              win = win.rearrange("(a two) d -> a (two d)", two=2)
                          subs = []  # per sub-gather: (tile, a, b, q, tick)
                          if abl != "compute":
                              for a in range(0, nch, GCH):
                                  b = min(a + GCH, nch)
                                  Msub = gpool.tile([128, (b - a) * EL], BF16, tag="gat")
                                  q = qstate["next"]
                                  qstate["next"] = (q + 1) % NQ
                                  qstate["cnt"][q] += 1
                                  gi = nc.gpsimd.dma_gather(
                                      out_ap=Msub[:].rearrange("p (t d) -> p t d", d=EL),
                                      in_ap=win,
                                      idxs_ap=idx_sb[:, a * 8 : b * 8],
                                      num_idxs=(b - a) * 128,
                                      num_idxs_reg=(b - a) * 128,
                                      elem_size=EL,
                                      single_packet=SP,
                                      queue_num=q,
                                  )
                                  if MANUAL_SYNC:
                                      gi.then_inc(qsems[q], 16)
                                  subs.append((Msub, a, b, q, qstate["cnt"][q]))
                          if abl in ("gather", "compute", "ehalf", "edouble"):
                              continue
                          t = 0
                          si = -1
                          Mcur = None
                          for g, ng in gl:
                              if g not in pst:
                                  pst[g] = aps.tile([128, D], F32, tag="agg", name=f"agg{g}")
                              for k in range(ng):
                                  if si < 0 or t >= subs[si][2]:
                                      si += 1
                                      Mcur = subs[si][0]
                                      if MANUAL_SYNC:
                                          nc.tensor.wait_ge(qsems[subs[si][3]], 16 * subs[si][4])
                                  S = ident[:] if abl == "nosel" else sbig_sb[:, t * 128 : (t + 1) * 128]
                                  nc.tensor.matmul(
                                      out=pst[g][:],
                                      lhsT=S,
                                      rhs=Mcur[:, (t - subs[si][1]) * D : (t - subs[si][1] + 1) * D],
                                      start=(ch == first_chunk[g]),
                                      stop=(ch == last_chunk[g]),
                                  )
                                  t += 1
                                  ch += 1
                          # epilogue for any group whose last chunk has been issued
                          done = [g for g in list(pst) if last_chunk[g] < ch]
                          for g in done:
                              h_t = opool.tile([128, D], BF16, tag="h")
                              if not act:
                                  nc.vector.tensor_scalar(
                                      out=h_t[:], in0=pst[g][:],
                                      scalar1=scal[:, g : g + 1], scalar2=None,
                                      op0=AluOp.mult,
                                  )
                              elif bias_tile is None:
                                  nc.vector.tensor_scalar(
                                      out=h_t[:], in0=pst[g][:],
                                      scalar1=scal[:, g : g + 1], scalar2=0.0,
                                      op0=AluOp.mult, op1=AluOp.max,
                                  )
                              else:
                                  tmp = epool.tile([128, D], F32, tag="tmp")
                                  nc.vector.tensor_scalar(
                                      out=tmp[:], in0=pst[g][:],
                                      scalar1=scal[:, g : g + 1], scalar2=None,
                                      op0=AluOp.mult,
                                  )
                                  nc.vector.tensor_tensor(
                                      out=tmp[:], in0=tmp[:], in1=bias_tile[:], op=AluOp.add
                                  )
                                  nc.vector.tensor_scalar(
                                      out=h_t[:], in0=tmp[:], scalar1=0.0,
                                      scalar2=None, op0=AluOp.max,
                                  )
                              out_cb(g, h_t)
                              del pst[g]

              # conv1: h1 = relu(r * agg + b1)  [r = dinv^2], store h1 to H1BUF
              def out1(g, h):
                  nc.sync.dma_start(H1BUF[g * 128 : (g + 1) * 128, :], h[:])

              if stage >= 3:
                  aggregate(H0F, r_, bias_tiles.get("b1"), out1)

              # ---- AG2: exchange h1' directly (W2 applied post-aggregation)
              if stage >= 5 and collectives:
                  nc.gpsimd.collective_compute(
                      "AllGather", AluOp.bypass, replica_groups=replica,
                      ins=[H1BUF.opt()], outs=[H1F.opt()],
                  )

              # ---- conv2 aggregation + W2 + classifier fused
              with tc.tile_pool(name="w2_ps", bufs=1, space="PSUM") as w2ps, \
                   tc.tile_pool(name="clf_ps", bufs=1, space="PSUM") as clps, \
                   tc.tile_pool(name="tp_ps", bufs=1, space="PSUM") as tpps, \
                   tc.tile_pool(name="h2t", bufs=4) as h2tp, \
                   tc.tile_pool(name="h2s", bufs=2) as h2sp, \
                   tc.tile_pool(name="clf_out", bufs=4) as clop:

                  def out2(g, t_bf):
                      # t_bf = dinv_g * agg2  (bf16, no act). h2 = relu(t @ W2);
                      # out = h2 @ Wc + bc
                      pt = w2ps.tile([128, D], F32, tag="w2o")
                      for b in range(2):
                          tp = tpps.tile([128, 128], BF16, tag="tp")
                          nc.tensor.transpose(
                              out=tp[:], in_=t_bf[:, b * 128 : (b + 1) * 128], identity=ident[:]
                          )
                          tT = h2tp.tile([128, 128], BF16, tag="tT")
                          nc.vector.tensor_copy(tT[:], tp[:])
                          nc.tensor.matmul(
                              out=pt[:],
                              lhsT=tT[:],
                              rhs=W2bf[:, b * D : (b + 1) * D],
                              start=(b == 0),
                              stop=(b == 1),
                          )
                      h2 = h2sp.tile([128, D], BF16, tag="h2")
                      if "b2" in bias_tiles:
                          tmp2 = h2sp.tile([128, D], F32, tag="tmp2")
                          nc.vector.tensor_tensor(
                              out=tmp2[:], in0=pt[:], in1=bias_tiles["b2"][:], op=AluOp.add
                          )
                          nc.vector.tensor_scalar(
                              out=h2[:], in0=tmp2[:], scalar1=0.0, scalar2=None, op0=AluOp.max
                          )
                      else:
                          nc.vector.tensor_scalar(
                              out=h2[:], in0=pt[:], scalar1=0.0, scalar2=None, op0=AluOp.max
                          )
                      cl = clps.tile([128, C], F32, tag="clf")
                      for b in range(2):
                          tp = tpps.tile([128, 128], BF16, tag="tp")
                          nc.tensor.transpose(
                              out=tp[:], in_=h2[:, b * 128 : (b + 1) * 128], identity=ident[:]
                          )
                          h2T = h2tp.tile([128, 128], BF16, tag="tT")
                          nc.vector.tensor_copy(h2T[:], tp[:])
                          nc.tensor.matmul(
                              out=cl[:],
                              lhsT=h2T[:],
                              rhs=Wcbf[:, b * C : (b + 1) * C],
                              start=(b == 0),
                              stop=(b == 1),
                          )
                      co = clop.tile([128, C], F32, tag="co")
                      if "bc" in bias_tiles:
                          nc.vector.tensor_tensor(
                              out=co[:], in0=cl[:], in1=bias_tiles["bc"][:], op=AluOp.add
                          )
                      else:
                          nc.vector.tensor_copy(co[:], cl[:])
                      nc.sync.dma_start(out_d[g * 128 : (g + 1) * 128, :], co[:])

                  if stage >= 6:
                      aggregate(H1F, dinv, None, out2, psum_bufs=5, act=False)

    nc.compile()
    return nc


# ----------------------------------------------------------------------------
# Entry point
# ----------------------------------------------------------------------------
_CACHE = {}


def _get_compiled(x, edge_index, edge_weight, b1, b2, bc):
    cores, meta = _preprocess(x, edge_index, edge_weight)
    has_b1 = bool(np.any(b1))
    has_b2 = bool(np.any(b2))
    has_bc = bool(np.any(bc))
    key = repr((meta["G"], meta["MAXD"], meta["TOTCH"], has_b1, has_b2, has_bc, meta["call_meta"]))
    nc = _CACHE.get(key)
    if nc is None:
        nc = _build(meta, has_b1, has_b2, has_bc)
        _CACHE[key] = nc
    return nc, cores, meta


def make_in_maps(inputs, cores, meta):
    x = np.asarray(inputs["x"], np.float32)
    W1 = np.asarray(inputs["W1"], np.float32)
    W2m = np.asarray(inputs["W2"], np.float32)
    Wc = np.asarray(inputs["Wc"], np.float32)
    b1 = np.asarray(inputs["b1"], np.float32)
    b2 = np.asarray(inputs["b2"], np.float32)
    bc = np.asarray(inputs["bc"], np.float32)
    iota = np.broadcast_to(np.arange(128, dtype=np.float32), (128, 128))
    in_maps = []
    for r in range(R):
        DEST = cores[r]["DEST"]
        WV = cores[r]["WV"]
        sbig = (
            (DEST[:, :, None] == np.arange(128, dtype=np.float32)) * WV[:, :, None]
        ).astype(ml_dtypes.bfloat16).reshape(128, -1)
        m = dict(
            xT=cores[r]["xT"],
            W1=W1,
            W2m=W2m,
            Wc=Wc,
            IDX16=cores[r]["IDX16"],
            SBIG=np.ascontiguousarray(sbig),
            WELL=cores[r]["WELL"],
            IOTA=np.ascontiguousarray(iota).astype(ml_dtypes.bfloat16),
            IOTAC=np.arange(128, dtype=np.float32).reshape(128, 1),
        )
        if np.any(b1):
            m["b1b"] = np.ascontiguousarray(np.broadcast_to(b1, (128, D))).astype(np.float32)
        if np.any(b2):
            m["b2b"] = np.ascontiguousarray(np.broadcast_to(b2, (128, D))).astype(np.float32)
        if np.any(bc):
            m["bcb"] = np.ascontiguousarray(np.broadcast_to(bc, (128, C))).astype(np.float32)
        in_maps.append(m)
    return in_maps


def unpack_output(results, meta):
    out = np.empty((N, C), np.float32)
    for r in range(R):
        out[r * NSH : (r + 1) * NSH] = results[r]["out"][meta["pcols"][r]]
    return out


def kernel(**inputs):
    nc, cores, meta = _get_compiled(
        inputs["x"], inputs["edge_index"], inputs["edge_weight"],
        inputs["b1"], inputs["b2"], inputs["bc"],
    )
    in_maps = make_in_maps(inputs, cores, meta)
    res = bass_utils.run_bass_kernel_spmd(nc, in_maps, core_ids=list(range(R)))
    return unpack_output(res.results, meta)

